# revision 1
# baseline (speedup 1.0000x reference)
"""Trainium2 Bass kernel: gated causal self-attention (GQA + partial RoPE).

Reference computation (per batch):
    q,k,v = x@Wq, x@Wk, x@Wv  (heads split, partial RoPE on first R dims)
    att = softmax(causal(q k^T / sqrt(D)))
    att = att * (att >= sigmoid(gate))          # post-softmax threshold gate
    y = (att @ v) @ Wo

Sharding over 8 NeuronCores: core = 4*b + g where b in {0,1} is the batch
(data parallel) and g in {0..3} is the KV-head group (tensor parallel:
Wq/Wk/Wv column-sharded, Wo row-sharded; gate sharded with heads).  Each
core computes a partial y^T (C x T); the host sums the 4 group partials
per batch and transposes.  The TxT score tensor never leaves a core.

On-chip layout: everything is computed transposed (qT/kT are (D,T),
scores are S^T = (s,t)) so that
  - softmax denominator = ones-matmul accumulation (and it lands
    partition-broadcast, exactly what the gate compare needs),
  - att@v needs no transposes: out^T accumulates with v-natural tiles as
    the stationary operand and gated exp(S^T) moving,
  - the output projection consumes out^T directly and emits y^T.

Precision split: the threshold-sensitive path (x, Wq, Wk, q^T, k^T, S^T)
runs float32r (FP22 multiply, FP32 accumulate, full PE rate); the
post-exp path (exp tiles, ones, v, Wo, out^T) runs float16, which turns
on Fast Weight Load for those matmuls and the DVE 2x mode for the
gating, at ~5e-4 relative cost on a purely linear/compare path.
exp() skips max-subtraction (scores are O(5), exp fits f16 range).
RoPE's rotate-half uses partition-shifted single-input copies plus
partition-aligned tensor_tensor ops; q-RoPE is batched across heads
with stride-0 broadcast APs for cos/sin.
"""

import numpy as np

import concourse.bass as bass
import concourse.tile as tile
from concourse import bacc, mybir
from concourse.alu_op_type import AluOpType
from concourse.bass_utils import run_bass_kernel_spmd

# Problem shapes (hardcoded per contract)
B, T, C = 2, 2048, 2048
H, HKV, D = 16, 4, 128
R = 64
NCORE = 8
G = 4            # tensor-parallel degree over KV heads
HL = H // G      # 4 local q heads per core
DL = HL * D      # 512 local q dims per core
SCALE = float(D) ** -0.5

F32 = mybir.dt.float32
F32R = mybir.dt.float32r
F16 = mybir.dt.float16
EXP = mybir.ActivationFunctionType.Exp

TB = 512                 # t-block width
NTB = T // TB            # 4
NCT = C // 128           # 16 contraction tiles
CQ = 4                   # c-tiles per xs chunk
NCHUNK = NCT // CQ       # 4 chunks
GB = 2                   # gating batch: s-tiles per DVE op

# packed f32 constant-tile column offsets: eye | thr
EYE0, THR0 = 0, 128
CONST_W = 128 + HL
# f16 mask tile: 4 diagonal masks (one per dpos) then a 128-wide ones block
ONES0 = 4 * TB
MSKS_W = 4 * TB + 128


def build():
    nc = bacc.Bacc("TRN2", target_bir_lowering=False, debug=False)

    xT = nc.dram_tensor("xT", [C, T], F32R, kind="ExternalInput").ap()
    wq = nc.dram_tensor("wq", [C, DL], F32R, kind="ExternalInput").ap()
    wk = nc.dram_tensor("wk", [C, D], F32R, kind="ExternalInput").ap()
    wv = nc.dram_tensor("wv", [C, D], F32R, kind="ExternalInput").ap()
    wo = nc.dram_tensor("wo", [DL, C], F16, kind="ExternalInput").ap()
    msks = nc.dram_tensor("msks", [128, MSKS_W], F16, kind="ExternalInput").ap()
    cs = nc.dram_tensor("cs", [R, T], F32, kind="ExternalInput").ap()
    sn = nc.dram_tensor("sn", [R, T], F32, kind="ExternalInput").ap()
    cst = nc.dram_tensor("cst", [128, CONST_W], F32, kind="ExternalInput").ap()
    ypT = nc.dram_tensor("ypT", [C, T], F32, kind="ExternalOutput").ap()

    with tile.TileContext(nc) as tc:
        with (
            tc.tile_pool(name="persist", bufs=1) as persist,
            tc.tile_pool(name="wpool", bufs=1) as wpool,
            tc.tile_pool(name="xpool", bufs=2) as xpool,
            tc.tile_pool(name="espool", bufs=2) as espool,
            tc.tile_pool(name="blk", bufs=2) as blk,
            tc.tile_pool(name="small", bufs=2) as small,
            tc.tile_pool(name="psum", bufs=1, space="PSUM") as psum,
        ):
            # ---- persistent SBUF ----
            kt = persist.tile([128, T], F32R)    # k^T (D x T), rope applied
            vn = persist.tile([128, T], F16)     # v natural; s-tile i at cols [128i,128i+128)
            cs_sb = persist.tile([R, T], F32)    # cos^T
            sn_sb = persist.tile([R, T], F32)    # sign-fixed sin^T: [-sinT[0:32] ; sinT[32:64]]
            msks_sb = persist.tile([128, MSKS_W], F16)
            cst_sb = persist.tile([128, CONST_W], F32)
            nc.sync.dma_start(cs_sb[:], cs)
            nc.sync.dma_start(sn_sb[:], sn)
            nc.sync.dma_start(msks_sb[:], msks)
            nc.sync.dma_start(cst_sb[:], cst)
            eye_sb = cst_sb[:, EYE0 : EYE0 + 128]
            thr_sb = cst_sb[:, THR0 : THR0 + HL]
            ones_sb = msks_sb[:, ONES0 : ONES0 + 128]

            # ---- weights (xs block 0 loads first, wo is deferred) ----
            wq_sb = wpool.tile([128, NCT, DL], F32R, tag="wq", name="wq_sb")
            wk_sb = wpool.tile([128, NCT, D], F32R, tag="wk", name="wk_sb")
            wv_sb = wpool.tile([128, NCT, D], F32R, tag="wv", name="wv_sb")
            wo_sb = wpool.tile([128, HL, C], F16, tag="wo", name="wo_sb")
            xs0_chunks = []
            for ch in range(NCHUNK):
                xs = xpool.tile([128, CQ, TB], F32R, tag="xs", name=f"xs_0_{ch}")
                for ci in range(CQ):
                    c = ch * CQ + ci
                    nc.sync.dma_start(xs[:, ci, :], xT[128 * c : 128 * (c + 1), 0:TB])
                xs0_chunks.append(xs)
            for c in range(NCT):
                csl = slice(128 * c, 128 * (c + 1))
                nc.sync.dma_start(wq_sb[:, c, :], wq[csl, :])
                nc.sync.dma_start(wk_sb[:, c, :], wk[csl, :])
                nc.sync.dma_start(wv_sb[:, c, :], wv[csl, :])

            def rope(th, dcols, tcols):
                """In-place partial RoPE on rows 0:R of region th[:, dcols].

                rotate-half via two partition-shifted single-input copies
                (legal on ACT), then partition-aligned tensor_tensor ops:
                  out[0:64] = q[0:64]*cos + rot*sin_signed
                with rot = [q[32:64]; q[0:32]], sin_signed = [-sin_lo; sin_hi].
                """
                hw = R // 2  # 32
                rot = small.tile([R, HL * TB], F32R, tag="ropeq", bufs=1, name="rope_rot")
                nc.scalar.copy(rot[0:hw, 0:TB], th[hw:R, dcols])
                nc.scalar.copy(rot[hw:R, 0:TB], th[0:hw, dcols])
                nc.vector.tensor_tensor(th[0:R, dcols], th[0:R, dcols], cs_sb[:, tcols], op=AluOpType.mult)
                nc.vector.tensor_tensor(rot[:, 0:TB], rot[:, 0:TB], sn_sb[:, tcols], op=AluOpType.mult)
                nc.vector.tensor_tensor(th[0:R, dcols], th[0:R, dcols], rot[:, 0:TB], op=AluOpType.add)

            def rope_q(qtb, tcols):
                """Batched RoPE over all HL head slices of qtb (same t-range),
                broadcasting cos/sin across the head dim with stride-0 APs."""
                hw = R // 2
                W = HL * TB
                rot = small.tile([R, W], F32R, tag="ropeq", bufs=1, name="ropeq_rot")
                nc.scalar.copy(rot[0:hw, :], qtb[hw:R, :])
                nc.scalar.copy(rot[hw:R, :], qtb[0:hw, :])
                qv = qtb[0:R, :].rearrange("p (r n) -> p r n", r=HL)
                rv = rot[:].rearrange("p (r n) -> p r n", r=HL)
                cb = cs_sb[:, tcols][:, None, :].broadcast_to([R, HL, TB])
                sb = sn_sb[:, tcols][:, None, :].broadcast_to([R, HL, TB])
                nc.vector.tensor_tensor(qv, qv, cb, op=AluOpType.mult)
                nc.vector.tensor_tensor(rv, rv, sb, op=AluOpType.mult)
                nc.vector.tensor_tensor(qv, qv, rv, op=AluOpType.add)

            # ---- main fully-unrolled t-block loop ----
            for j in range(NTB):
                tsl = slice(j * TB, (j + 1) * TB)

                # --- projections for block j ---
                if j == 0:
                    xs_chunks = xs0_chunks
                else:
                    xs_chunks = []
                    for ch in range(NCHUNK):
                        xs = xpool.tile([128, CQ, TB], F32R, tag="xs", name=f"xs_{j}_{ch}")
                        for ci in range(CQ):
                            c = ch * CQ + ci
                            nc.sync.dma_start(xs[:, ci, :], xT[128 * c : 128 * (c + 1), tsl])
                        xs_chunks.append(xs)

                # All 6 projection accumulators open at once; consume each
                # xs chunk fully before the next (xpool bufs=2 then suffices).
                qtb = blk.tile([128, HL * TB], F32R, tag="qtb", name=f"qtb_{j}")
                qps = [
                    psum.tile([128, TB], F32, tag="mm", bufs=4, name=f"qp_{j}_{h}")
                    for h in range(HL)
                ]
                kp = psum.tile([128, TB], F32, tag="acc", bufs=4, name=f"kp_{j}")
                vp = psum.tile([128, TB], F32, tag="acc", bufs=4, name=f"vp_{j}")
                groups = [(qps[h], wq_sb, 128 * h, 128) for h in range(HL)]
                groups += [(kp, wk_sb, 0, D), (vp, wv_sb, 0, D)]
                for ch in range(NCHUNK):
                    for gp, w_sb, col0, ncols in groups:
                        for ci in range(CQ):
                            c = ch * CQ + ci
                            nc.tensor.matmul(
                                gp[:],
                                w_sb[:, c, col0 : col0 + ncols],
                                xs_chunks[ch][:, ci, :],
                                start=(c == 0),
                                stop=(c == NCT - 1),
                            )
                for h in range(HL):
                    nc.scalar.copy(qtb[:, TB * h : TB * (h + 1)], qps[h][:])
                rope_q(qtb, tsl)
                nc.scalar.copy(kt[:, tsl], kp[:])
                rope(kt, tsl, tsl)
                vt_tmp = small.tile([128, TB], F32, tag="vt", bufs=1, name=f"vt_{j}")
                nc.scalar.copy(vt_tmp[:], vp[:])
                for u in range(TB // 128):
                    tp = psum.tile([128, 128], F32, tag="acc", bufs=4, name=f"tp_{j}_{u}")
                    nc.tensor.transpose(tp[:], vt_tmp[:, 128 * u : 128 * (u + 1)], eye_sb)
                    s_idx = j * (TB // 128) + u
                    nc.vector.tensor_copy(vn[:, 128 * s_idx : 128 * (s_idx + 1)], tp[:])

                if j == 0:
                    # wo is first needed by block 0's output projection; loading
                    # it here overlaps the DMA with block 0 compute instead of
                    # delaying the first matmul.
                    for d in range(HL):
                        nc.sync.dma_start(wo_sb[:, d, :], wo[128 * d : 128 * (d + 1), :])

                # --- attention for block j, all local heads ---
                nst = 4 * j + 4  # causal: s-tiles 0 .. 4j+3
                ytb = blk.tile([128, HL * TB], F16, tag="ytb", name=f"ytb_{j}")
                for h in range(HL):
                    qsl = slice(TB * h, TB * (h + 1))
                    esb = espool.tile([128, nst * TB], F16, tag="es", name=f"es_{j}_{h}")
                    # phase A: scores + exp (+ causal masks on the 4 diagonal tiles)
                    for i in range(nst):
                        ssl = slice(128 * i, 128 * (i + 1))
                        sp = psum.tile([128, TB], F32, tag="mm", bufs=4, name=f"sp_{j}_{h}_{i}")
                        nc.tensor.matmul(
                            sp[:], kt[:, ssl], qtb[:, qsl], start=True, stop=True
                        )
                        es = esb[:, TB * i : TB * (i + 1)]
                        nc.scalar.activation(es, sp[:], EXP, scale=SCALE)
                        dpos = i - 4 * j
                        if dpos >= 0:
                            # diagonal tile: mask dpos = [zeros(128*dpos) | tri | ones]
                            nc.vector.tensor_tensor(
                                es, es, msks_sb[:, TB * dpos : TB * (dpos + 1)],
                                op=AluOpType.mult,
                            )
                    # phase B: denominator (dense PE accumulation, f16+FWL)
                    dn = psum.tile([128, TB], F32, tag="acc", bufs=4, name=f"dn_{j}_{h}")
                    for i in range(nst):
                        nc.tensor.matmul(
                            dn[:], ones_sb, esb[:, TB * i : TB * (i + 1)],
                            start=(i == 0), stop=(i == nst - 1),
                        )
                    # phase C: threshold row (f16) and 1/denom (fast NR reciprocal)
                    work = small.tile([128, TB], F32, tag="work", bufs=2, name=f"work_{j}_{h}")
                    cwork = small.tile([128, TB], F16, tag="cwork", bufs=2, name=f"cwork_{j}_{h}")
                    cthr = cwork[:]
                    rden = work[:]
                    nc.vector.tensor_scalar_mul(cthr, dn[:], thr_sb[:, h : h + 1])
                    nc.vector.reciprocal_approx_fast(out=rden, in_=dn[:])
                    # phase D: batched gating, GB tiles per DVE op (f16, 2x mode)
                    for g0 in range(0, nst, GB):
                        gn = min(GB, nst - g0)
                        ev = esb[:, TB * g0 : TB * (g0 + gn)].rearrange(
                            "p (r n) -> p r n", r=gn
                        )
                        cb = cthr[:, None, :].broadcast_to([128, gn, TB])
                        msk = small.tile([128, GB * TB], F16, tag="msk", bufs=2, name=f"msk_{j}_{h}_{g0}")
                        mv = msk[:, 0 : TB * gn].rearrange("p (r n) -> p r n", r=gn)
                        nc.vector.tensor_tensor(mv, ev, cb, op=AluOpType.is_ge)
                        nc.vector.tensor_tensor(ev, ev, mv, op=AluOpType.mult)
                    # phase E: att @ v (dense, f16+FWL), then normalize
                    yp = psum.tile([128, TB], F32, tag="acc", bufs=4, name=f"yp_{j}_{h}")
                    for i in range(nst):
                        nc.tensor.matmul(
                            yp[:], vn[:, 128 * i : 128 * (i + 1)], esb[:, TB * i : TB * (i + 1)],
                            start=(i == 0), stop=(i == nst - 1),
                        )
                    nc.vector.tensor_tensor(ytb[:, qsl], yp[:], rden, op=AluOpType.mult)

                # --- output projection for block j (f16 + FWL) ---
                for co in range(C // 128):
                    op = psum.tile([128, TB], F32, tag="mm", bufs=4, name=f"op_{j}_{co}")
                    for d in range(HL):
                        nc.tensor.matmul(
                            op[:],
                            wo_sb[:, d, 128 * co : 128 * (co + 1)],
                            ytb[:, TB * d : TB * (d + 1)],
                            start=(d == 0),
                            stop=(d == HL - 1),
                        )
                    stg = small.tile([128, TB], F32, tag="stg", bufs=2, name=f"stg_{j}_{co}")
                    nc.scalar.copy(stg[:], op[:])
                    nc.sync.dma_start(ypT[128 * co : 128 * (co + 1), tsl], stg[:])

    nc.compile()
    return nc


_NC_CACHE = None


def _get_nc():
    global _NC_CACHE
    if _NC_CACHE is None:
        _NC_CACHE = build()
    return _NC_CACHE


def make_in_maps(x, cos, sin, Wq, Wk, Wv, Wo, gate):
    x = np.asarray(x, np.float32)
    cos = np.asarray(cos, np.float32)
    sin = np.asarray(sin, np.float32)
    Wq = np.asarray(Wq, np.float32)
    Wk = np.asarray(Wk, np.float32)
    Wv = np.asarray(Wv, np.float32)
    Wo = np.asarray(Wo, np.float32)
    gate = np.asarray(gate, np.float32)

    hw = R // 2
    cosT = np.ascontiguousarray(cos.T)  # (R, T)
    sinT = sin.T
    sn_signed = np.ascontiguousarray(np.concatenate([-sinT[0:hw], sinT[hw:R]], axis=0))
    thr_full = 1.0 / (1.0 + np.exp(-gate))  # sigmoid, (H,)
    tri = np.triu(np.ones((128, 128), np.float32))  # valid: s <= t
    cst_base = np.zeros((128, CONST_W), np.float32)
    cst_base[:, EYE0 : EYE0 + 128] = np.eye(128, dtype=np.float32)
    # f16 masks: for the diagonal s-tile at dpos, cols [0,128*dpos) invalid
    # (zeros), a 128-wide triangle at [128*dpos, ...), ones after.
    msks = np.zeros((128, MSKS_W), np.float16)
    for dpos in range(4):
        m = np.zeros((128, TB), np.float32)
        m[:, 128 * dpos : 128 * (dpos + 1)] = tri
        m[:, 128 * (dpos + 1) :] = 1.0
        msks[:, TB * dpos : TB * (dpos + 1)] = m
    msks[:, ONES0 : ONES0 + 128] = 1.0

    in_maps = []
    for core in range(NCORE):
        b, g = divmod(core, G)
        cst = cst_base.copy()
        cst[:, THR0 : THR0 + HL] = thr_full[HL * g : HL * (g + 1)]
        in_maps.append(
            {
                "xT": np.ascontiguousarray(x[b].T),
                "wq": np.ascontiguousarray(Wq[:, DL * g : DL * (g + 1)]),
                "wk": np.ascontiguousarray(Wk[:, D * g : D * (g + 1)]),
                "wv": np.ascontiguousarray(Wv[:, D * g : D * (g + 1)]),
                "wo": np.ascontiguousarray(Wo[DL * g : DL * (g + 1), :].astype(np.float16)),
                "msks": msks,
                "cs": cosT,
                "sn": sn_signed,
                "cst": cst,
            }
        )
    return in_maps


def run(inputs, trace=False, **kw):
    """Run on 8 NeuronCores; returns (y_full, BassKernelResults)."""
    nc = _get_nc()
    in_maps = make_in_maps(**inputs)
    res = run_bass_kernel_spmd(nc, in_maps, core_ids=list(range(NCORE)), trace=trace, **kw)
    y = np.zeros((B, T, C), np.float32)
    for core in range(NCORE):
        b = core // G
        y[b] += res.results[core]["ypT"].T
    return y, res


def kernel(**inputs) -> np.ndarray:
    y, _ = run(inputs)
    return y



# revision 4
# speedup vs baseline: 1.1654x; 1.1654x over previous
"""Trainium2 Bass kernel: gated causal self-attention (GQA + partial RoPE).

Reference computation (per batch):
    q,k,v = x@Wq, x@Wk, x@Wv  (heads split, partial RoPE on first R dims)
    att = softmax(causal(q k^T / sqrt(D)))
    att = att * (att >= sigmoid(gate))          # post-softmax threshold gate
    y = (att @ v) @ Wo

Sharding over 8 NeuronCores: core = 4*b + g where b in {0,1} is the batch
(data parallel) and g in {0..3} is the KV-head group (tensor parallel:
Wq/Wk/Wv column-sharded, Wo row-sharded; gate sharded with heads).  Each
core computes a partial y^T (C x T) in f16; the host sums the 4 group
partials per batch (upcast to f32) and transposes.  The TxT score tensor
never leaves a core.

On-chip layout: everything is computed transposed (qT/kT are (D,T),
scores are S^T = (s,t)) so that
  - softmax denominator = ones-matmul accumulation (and it lands
    partition-broadcast, exactly what the gate compare needs),
  - att@v needs no transposes: out^T accumulates with v-natural tiles as
    the stationary operand and gated exp(S^T) moving,
  - the output projection consumes out^T directly and emits y^T.

Schedule: one-block software pipeline.  Per block j we emit the
projections + RoPE of block j and then the attention + output projection
of block j-1, so the RoPE chain (ACT/DVE) and the gating chain (DVE)
always have dense PE work (next block's projections / previous block's
attention) to hide under.  This also keeps the PE free of >3.4us idle
gaps, which would re-throttle its clock (HAM) to half rate.

PSUM (8 banks): tag "a" bufs=4 hosts the 4 q-head projection
accumulators and, later in priority order, the output-projection tiles;
tag "b" bufs=2 hosts k/v accumulators and the denominator / att@v
accumulators; tag "c" bufs=2 hosts score tiles and the v-transpose.

Diagonal (causal-boundary) score tiles only compute/exp the causally
reachable column range; the denominator and att@v matmuls restrict their
moving operands to the same range, so no masked-out work hits the PE.

Precision split: the threshold-sensitive path (x, Wq, Wk, q^T, k^T, S^T)
runs float32r (FP22 multiply, FP32 accumulate, full PE rate); the
post-exp path (exp tiles, ones, v, Wo, out^T) runs float16.  The y^T
partials are emitted in f16 (host accumulates in f32).
"""

import numpy as np

import concourse.bass as bass
import concourse.tile as tile
from concourse import bacc, mybir
from concourse.alu_op_type import AluOpType
from concourse.bass_utils import run_bass_kernel_spmd

# Problem shapes (hardcoded per contract)
B, T, C = 2, 2048, 2048
H, HKV, D = 16, 4, 128
R = 64
NCORE = 8
G = 4            # tensor-parallel degree over KV heads
HL = H // G      # 4 local q heads per core
DL = HL * D      # 512 local q dims per core
SCALE = float(D) ** -0.5

F32 = mybir.dt.float32
F32R = mybir.dt.float32r
F16 = mybir.dt.float16
EXP = mybir.ActivationFunctionType.Exp

TB = 512                 # t-block width
NTB = T // TB            # 4
NCT = C // 128           # 16 contraction tiles
CQ = 4                   # c-tiles per xs chunk
NCHUNK = NCT // CQ       # 4 chunks

# packed f32 constant-tile column offsets: eye | thr
EYE0, THR0 = 0, 128
CONST_W = 128 + HL
# f16 mask tile: 128-wide causal triangle then a 128-wide ones block
TRI0, ONES0 = 0, 128
MSKS_W = 256


def build():
    nc = bacc.Bacc("TRN2", target_bir_lowering=False, debug=False)

    xT = nc.dram_tensor("xT", [C, T], F32R, kind="ExternalInput").ap()
    wq = nc.dram_tensor("wq", [C, DL], F32R, kind="ExternalInput").ap()
    wk = nc.dram_tensor("wk", [C, D], F32R, kind="ExternalInput").ap()
    wv = nc.dram_tensor("wv", [C, D], F32R, kind="ExternalInput").ap()
    wo = nc.dram_tensor("wo", [DL, C], F16, kind="ExternalInput").ap()
    msks = nc.dram_tensor("msks", [128, MSKS_W], F16, kind="ExternalInput").ap()
    cs = nc.dram_tensor("cs", [R, T], F32, kind="ExternalInput").ap()
    sn = nc.dram_tensor("sn", [R, T], F32, kind="ExternalInput").ap()
    cst = nc.dram_tensor("cst", [128, CONST_W], F32, kind="ExternalInput").ap()
    ypT = nc.dram_tensor("ypT", [C, T], F16, kind="ExternalOutput").ap()

    with tile.TileContext(nc) as tc:
        with (
            tc.tile_pool(name="persist", bufs=1) as persist,
            tc.tile_pool(name="wpool", bufs=1) as wpool,
            tc.tile_pool(name="xpool", bufs=2) as xpool,
            tc.tile_pool(name="espool", bufs=2) as espool,
            tc.tile_pool(name="blk", bufs=2) as blk,
            tc.tile_pool(name="small", bufs=2) as small,
            tc.tile_pool(name="stgp", bufs=4) as stgp,
            tc.tile_pool(name="psum", bufs=1, space="PSUM") as psum,
        ):
            # ---- persistent SBUF ----
            kt = persist.tile([128, T], F32R)    # k^T (D x T), rope applied
            vn = persist.tile([128, T], F16)     # v natural; s-tile i at cols [128i,128i+128)
            cs_sb = persist.tile([R, T], F32)    # cos^T
            sn_sb = persist.tile([R, T], F32)    # sign-fixed sin^T: [-sinT[0:32] ; sinT[32:64]]
            msks_sb = persist.tile([128, MSKS_W], F16)
            cst_sb = persist.tile([128, CONST_W], F32)
            nc.sync.dma_start(msks_sb[:], msks)
            nc.sync.dma_start(cst_sb[:], cst)
            nc.sync.dma_start(cs_sb[:], cs)
            nc.sync.dma_start(sn_sb[:], sn)
            eye_sb = cst_sb[:, EYE0 : EYE0 + 128]
            thr_sb = cst_sb[:, THR0 : THR0 + HL]
            tri_sb = msks_sb[:, TRI0 : TRI0 + 128]
            ones_sb = msks_sb[:, ONES0 : ONES0 + 128]

            # ---- weights, interleaved with block-0 x chunks so the first
            # projection matmuls start as soon as chunk 0 + its weights land
            wq_sb = wpool.tile([128, NCT, DL], F32R, tag="wq", name="wq_sb")
            wk_sb = wpool.tile([128, NCT, D], F32R, tag="wk", name="wk_sb")
            wv_sb = wpool.tile([128, NCT, D], F32R, tag="wv", name="wv_sb")
            wo_sb = wpool.tile([128, HL, C], F16, tag="wo", name="wo_sb")
            xs0_chunks = []
            for ch in range(NCHUNK):
                xs = xpool.tile([128, CQ, TB], F32R, tag="xs", name=f"xs_0_{ch}")
                for ci in range(CQ):
                    c = ch * CQ + ci
                    csl = slice(128 * c, 128 * (c + 1))
                    nc.sync.dma_start(xs[:, ci, :], xT[csl, 0:TB])
                    nc.sync.dma_start(wq_sb[:, c, :], wq[csl, :])
                    nc.sync.dma_start(wk_sb[:, c, :], wk[csl, :])
                    nc.sync.dma_start(wv_sb[:, c, :], wv[csl, :])
                xs0_chunks.append(xs)

            def rope(th, dcols, tcols):
                """In-place partial RoPE on rows 0:R of region th[:, dcols].

                rotate-half via two partition-shifted single-input copies
                (legal on ACT), then partition-aligned tensor_tensor ops:
                  out[0:64] = q[0:64]*cos + rot*sin_signed
                with rot = [q[32:64]; q[0:32]], sin_signed = [-sin_lo; sin_hi].
                """
                hw = R // 2  # 32
                rot = small.tile([R, HL * TB], F32R, tag="ropeq", bufs=1, name="rope_rot")
                nc.scalar.copy(rot[0:hw, 0:TB], th[hw:R, dcols])
                nc.scalar.copy(rot[hw:R, 0:TB], th[0:hw, dcols])
                nc.vector.tensor_tensor(th[0:R, dcols], th[0:R, dcols], cs_sb[:, tcols], op=AluOpType.mult)
                nc.vector.tensor_tensor(rot[:, 0:TB], rot[:, 0:TB], sn_sb[:, tcols], op=AluOpType.mult)
                nc.vector.tensor_tensor(th[0:R, dcols], th[0:R, dcols], rot[:, 0:TB], op=AluOpType.add)

            def rope_q(qtb, tcols):
                """Batched RoPE over all HL head slices of qtb (same t-range),
                broadcasting cos/sin across the head dim with stride-0 APs."""
                hw = R // 2
                W = HL * TB
                rot = small.tile([R, W], F32R, tag="ropeq", bufs=1, name="ropeq_rot")
                nc.scalar.copy(rot[0:hw, :], qtb[hw:R, :])
                nc.scalar.copy(rot[hw:R, :], qtb[0:hw, :])
                qv = qtb[0:R, :].rearrange("p (r n) -> p r n", r=HL)
                rv = rot[:].rearrange("p (r n) -> p r n", r=HL)
                cb = cs_sb[:, tcols][:, None, :].broadcast_to([R, HL, TB])
                sb = sn_sb[:, tcols][:, None, :].broadcast_to([R, HL, TB])
                nc.vector.tensor_tensor(qv, qv, cb, op=AluOpType.mult)
                nc.vector.tensor_tensor(rv, rv, sb, op=AluOpType.mult)
                nc.vector.tensor_tensor(qv, qv, rv, op=AluOpType.add)

            qtbs = {}

            def proj_block(j):
                """Projections, RoPE, and v-transpose for t-block j."""
                tsl = slice(j * TB, (j + 1) * TB)
                if j == 0:
                    xs_chunks = xs0_chunks
                else:
                    xs_chunks = []
                    for ch in range(NCHUNK):
                        xs = xpool.tile([128, CQ, TB], F32R, tag="xs", name=f"xs_{j}_{ch}")
                        for ci in range(CQ):
                            c = ch * CQ + ci
                            nc.sync.dma_start(xs[:, ci, :], xT[128 * c : 128 * (c + 1), tsl])
                        xs_chunks.append(xs)

                # All 6 projection accumulators open at once; consume each
                # xs chunk fully before the next (xpool bufs=2 then suffices).
                qtb = blk.tile([128, HL * TB], F32R, tag="qtb", name=f"qtb_{j}")
                qtbs[j] = qtb
                qps = [
                    psum.tile([128, TB], F32, tag="a", bufs=4, name=f"qp_{j}_{h}")
                    for h in range(HL)
                ]
                kp = psum.tile([128, TB], F32, tag="b", bufs=2, name=f"kp_{j}")
                vp = psum.tile([128, TB], F32, tag="b", bufs=2, name=f"vp_{j}")
                groups = [(qps[h], wq_sb, 128 * h, 128) for h in range(HL)]
                groups += [(kp, wk_sb, 0, D), (vp, wv_sb, 0, D)]
                for ch in range(NCHUNK):
                    for gp, w_sb, col0, ncols in groups:
                        for ci in range(CQ):
                            c = ch * CQ + ci
                            nc.tensor.matmul(
                                gp[:],
                                w_sb[:, c, col0 : col0 + ncols],
                                xs_chunks[ch][:, ci, :],
                                start=(c == 0),
                                stop=(c == NCT - 1),
                            )
                for h in range(HL):
                    nc.scalar.copy(qtb[:, TB * h : TB * (h + 1)], qps[h][:])
                rope_q(qtb, tsl)
                nc.scalar.copy(kt[:, tsl], kp[:])
                rope(kt, tsl, tsl)
                vt_tmp = small.tile([128, TB], F32, tag="vt", bufs=1, name=f"vt_{j}")
                nc.scalar.copy(vt_tmp[:], vp[:])
                for u in range(TB // 128):
                    tp = psum.tile([128, 128], F32, tag="c", bufs=2, name=f"tp_{j}_{u}")
                    nc.tensor.transpose(tp[:], vt_tmp[:, 128 * u : 128 * (u + 1)], eye_sb)
                    s_idx = j * (TB // 128) + u
                    nc.vector.tensor_copy(vn[:, 128 * s_idx : 128 * (s_idx + 1)], tp[:])

                if j == 0:
                    # wo is first needed by block 0's output projection;
                    # loading it here overlaps the DMA with compute.
                    for d in range(HL):
                        nc.sync.dma_start(wo_sb[:, d, :], wo[128 * d : 128 * (d + 1), :])

            def att_block(j):
                """Attention + output projection for t-block j (all local heads)."""
                tsl = slice(j * TB, (j + 1) * TB)
                qtb = qtbs.pop(j)
                nst = 4 * j + 4  # causal: s-tiles 0 .. 4j+3
                ytb = blk.tile([128, HL * TB], F16, tag="ytb", name=f"ytb_{j}")
                for h in range(HL):
                    qsl = slice(TB * h, TB * (h + 1))
                    esb = espool.tile([128, nst * TB], F16, tag="es", name=f"es_{j}_{h}")
                    # offsets: diagonal s-tile dpos only reaches t >= 128*dpos
                    offs = [max(0, (i - 4 * j) * 128) for i in range(nst)]
                    # phase A: scores + exp, restricted to the causally
                    # reachable range; tri-mask on the 128-wide diagonal block
                    for i in range(nst):
                        off = offs[i]
                        ssl = slice(128 * i, 128 * (i + 1))
                        sp = psum.tile([128, TB], F32, tag="c", bufs=2, name=f"sp_{j}_{h}_{i}")
                        nc.tensor.matmul(
                            sp[:, off:TB],
                            kt[:, ssl],
                            qtb[:, TB * h + off : TB * (h + 1)],
                            start=True,
                            stop=True,
                        )
                        es = esb[:, TB * i + off : TB * (i + 1)]
                        nc.scalar.activation(es, sp[:, off:TB], EXP, scale=SCALE)
                        if i - 4 * j >= 0:
                            nc.vector.tensor_tensor(
                                esb[:, TB * i + off : TB * i + off + 128],
                                esb[:, TB * i + off : TB * i + off + 128],
                                tri_sb,
                                op=AluOpType.mult,
                            )
                    # phase B: denominator (dense PE accumulation, f16)
                    dn = psum.tile([128, TB], F32, tag="b", bufs=2, name=f"dn_{j}_{h}")
                    for i in range(nst):
                        off = offs[i]
                        nc.tensor.matmul(
                            dn[:, off:TB], ones_sb, esb[:, TB * i + off : TB * (i + 1)],
                            start=(i == 0), stop=(i == nst - 1),
                        )
                    # phase C: threshold row (f16) and 1/denom (fast NR reciprocal)
                    work = small.tile([128, TB], F32, tag="work", bufs=2, name=f"work_{j}_{h}")
                    cwork = small.tile([128, TB], F16, tag="cwork", bufs=2, name=f"cwork_{j}_{h}")
                    cthr = cwork[:]
                    rden = work[:]
                    nc.vector.tensor_scalar_mul(cthr, dn[:], thr_sb[:, h : h + 1])
                    nc.vector.reciprocal_approx_fast(out=rden, in_=dn[:])
                    # phase D: gating es *= (es >= cthr); one big op over the
                    # full-width tiles, per-tile ops over the diagonal ones
                    # (their dead ranges are never written, so never read).
                    # sized for the largest single gating op (12 full tiles at
                    # j=3); per-op ranges index it from 0
                    msk = small.tile([128, 12 * TB], F16, tag="msk", bufs=1, name=f"msk_{j}_{h}")
                    gate_ranges = []
                    if j > 0:
                        gate_ranges.append((0, 4 * j))  # full tiles, as one op
                    for i in range(4 * j, nst):
                        gate_ranges.append((i, i + 1))
                    for i0, i1 in gate_ranges:
                        off = offs[i0]
                        gn = i1 - i0
                        if gn > 1:
                            ev = esb[:, TB * i0 : TB * i1].rearrange(
                                "p (r n) -> p r n", r=gn
                            )
                            mv = msk[:, 0 : TB * gn].rearrange(
                                "p (r n) -> p r n", r=gn
                            )
                            cb = cthr[:, None, :].broadcast_to([128, gn, TB])
                        else:
                            ev = esb[:, TB * i0 + off : TB * i1]
                            mv = msk[:, 0 : TB - off]
                            cb = cthr[:, off:TB]
                        nc.vector.tensor_tensor(mv, ev, cb, op=AluOpType.is_ge)
                        nc.vector.tensor_tensor(ev, ev, mv, op=AluOpType.mult)
                    # phase E: att @ v (dense, f16), then normalize
                    yp = psum.tile([128, TB], F32, tag="b", bufs=2, name=f"yp_{j}_{h}")
                    for i in range(nst):
                        off = offs[i]
                        nc.tensor.matmul(
                            yp[:, off:TB],
                            vn[:, 128 * i : 128 * (i + 1)],
                            esb[:, TB * i + off : TB * (i + 1)],
                            start=(i == 0), stop=(i == nst - 1),
                        )
                    nc.vector.tensor_tensor(ytb[:, qsl], yp[:], rden, op=AluOpType.mult)

                # --- output projection for block j (f16) ---
                for co in range(C // 128):
                    op = psum.tile([128, TB], F32, tag="a", bufs=4, name=f"op_{j}_{co}")
                    for d in range(HL):
                        nc.tensor.matmul(
                            op[:],
                            wo_sb[:, d, 128 * co : 128 * (co + 1)],
                            ytb[:, TB * d : TB * (d + 1)],
                            start=(d == 0),
                            stop=(d == HL - 1),
                        )
                    stg = stgp.tile([128, TB], F16, tag="stg", name=f"stg_{j}_{co}")
                    if co % 2 == 0:
                        nc.scalar.copy(stg[:], op[:])
                    else:
                        nc.vector.tensor_copy(stg[:], op[:])
                    nc.sync.dma_start(ypT[128 * co : 128 * (co + 1), tsl], stg[:])

            # ---- main loop: one-block software pipeline ----
            for j in range(NTB):
                proj_block(j)
                if j > 0:
                    att_block(j - 1)
            att_block(NTB - 1)

    nc.compile()
    return nc


_NC_CACHE = None


def _get_nc():
    global _NC_CACHE
    if _NC_CACHE is None:
        _NC_CACHE = build()
    return _NC_CACHE


def make_in_maps(x, cos, sin, Wq, Wk, Wv, Wo, gate):
    x = np.asarray(x, np.float32)
    cos = np.asarray(cos, np.float32)
    sin = np.asarray(sin, np.float32)
    Wq = np.asarray(Wq, np.float32)
    Wk = np.asarray(Wk, np.float32)
    Wv = np.asarray(Wv, np.float32)
    Wo = np.asarray(Wo, np.float32)
    gate = np.asarray(gate, np.float32)

    hw = R // 2
    cosT = np.ascontiguousarray(cos.T)  # (R, T)
    sinT = sin.T
    sn_signed = np.ascontiguousarray(np.concatenate([-sinT[0:hw], sinT[hw:R]], axis=0))
    thr_full = 1.0 / (1.0 + np.exp(-gate))  # sigmoid, (H,)
    cst_base = np.zeros((128, CONST_W), np.float32)
    cst_base[:, EYE0 : EYE0 + 128] = np.eye(128, dtype=np.float32)
    # f16 masks: 128-wide causal triangle (valid: s <= t) and a ones block
    msks = np.zeros((128, MSKS_W), np.float16)
    msks[:, TRI0 : TRI0 + 128] = np.triu(np.ones((128, 128), np.float32))
    msks[:, ONES0 : ONES0 + 128] = 1.0

    in_maps = []
    for core in range(NCORE):
        b, g = divmod(core, G)
        cst = cst_base.copy()
        cst[:, THR0 : THR0 + HL] = thr_full[HL * g : HL * (g + 1)]
        in_maps.append(
            {
                "xT": np.ascontiguousarray(x[b].T),
                "wq": np.ascontiguousarray(Wq[:, DL * g : DL * (g + 1)]),
                "wk": np.ascontiguousarray(Wk[:, D * g : D * (g + 1)]),
                "wv": np.ascontiguousarray(Wv[:, D * g : D * (g + 1)]),
                "wo": np.ascontiguousarray(Wo[DL * g : DL * (g + 1), :].astype(np.float16)),
                "msks": msks,
                "cs": cosT,
                "sn": sn_signed,
                "cst": cst,
            }
        )
    return in_maps


def run(inputs, trace=False, **kw):
    """Run on 8 NeuronCores; returns (y_full, BassKernelResults)."""
    nc = _get_nc()
    in_maps = make_in_maps(**inputs)
    res = run_bass_kernel_spmd(nc, in_maps, core_ids=list(range(NCORE)), trace=trace, **kw)
    y = np.zeros((B, T, C), np.float32)
    for core in range(NCORE):
        b = core // G
        y[b] += res.results[core]["ypT"].T.astype(np.float32)
    return y, res


def kernel(**inputs) -> np.ndarray:
    y, _ = run(inputs)
    return y


# revision 8
# speedup vs baseline: 1.2123x; 1.0403x over previous
"""Trainium2 Bass kernel: gated causal self-attention (GQA + partial RoPE).

Reference computation (per batch):
    q,k,v = x@Wq, x@Wk, x@Wv  (heads split, partial RoPE on first R dims)
    att = softmax(causal(q k^T / sqrt(D)))
    att = att * (att >= sigmoid(gate))          # post-softmax threshold gate
    y = (att @ v) @ Wo

Sharding over 8 NeuronCores: core = 4*b + g where b in {0,1} is the batch
(data parallel) and g in {0..3} is the KV-head group (tensor parallel:
Wq/Wk/Wv column-sharded, Wo row-sharded; gate sharded with heads).  Each
core computes a partial y^T (C x T) in f16; the host sums the 4 group
partials per batch (upcast to f32) and transposes.  The TxT score tensor
never leaves a core.

On-chip layout: everything is computed transposed (qT/kT are (D,T),
scores are S^T = (s,t)) so that
  - softmax denominator = ones-matmul accumulation (and it lands
    partition-broadcast, exactly what the gate compare needs),
  - att@v needs no transposes: out^T accumulates with v-natural tiles as
    the stationary operand and gated exp(S^T) moving,
  - the output projection consumes out^T directly and emits y^T.

Schedule: one-block software pipeline.  Per block j we emit the
projections + RoPE of block j and then the attention + output projection
of block j-1, so the RoPE chain (ACT/DVE) and the gating chain (DVE)
always have dense PE work (next block's projections / previous block's
attention) to hide under.  This also keeps the PE free of >3.4us idle
gaps, which would re-throttle its clock (HAM) to half rate.

PSUM (8 banks): tag "a" bufs=4 hosts the 4 q-head projection
accumulators and, later in priority order, the output-projection tiles;
tag "b" bufs=2 hosts k/v accumulators and the denominator / att@v
accumulators; tag "c" bufs=2 hosts score tiles and the v-transpose.

Diagonal (causal-boundary) score tiles only compute/exp the causally
reachable column range; the denominator and att@v matmuls restrict their
moving operands to the same range, so no masked-out work hits the PE.

Precision split: the threshold-sensitive path (x, Wq, Wk, q^T, k^T, S^T)
runs float32r (FP22 multiply, FP32 accumulate, full PE rate); the
post-exp path (exp tiles, ones, v, Wo, out^T) runs float16.  The y^T
partials are emitted in f16 (host accumulates in f32).
"""

import numpy as np

import concourse.bass as bass
import concourse.tile as tile
from concourse import bacc, mybir
from concourse.alu_op_type import AluOpType
from concourse.bass_utils import run_bass_kernel_spmd

# Problem shapes (hardcoded per contract)
B, T, C = 2, 2048, 2048
H, HKV, D = 16, 4, 128
R = 64
NCORE = 8
G = 4            # tensor-parallel degree over KV heads
HL = H // G      # 4 local q heads per core
DL = HL * D      # 512 local q dims per core
SCALE = float(D) ** -0.5

F32 = mybir.dt.float32
F32R = mybir.dt.float32r
F16 = mybir.dt.float16
EXP = mybir.ActivationFunctionType.Exp

TB = 512                 # t-block width
NTB = T // TB            # 4
NCT = C // 128           # 16 contraction tiles
CQ = 4                   # c-tiles per xs chunk
NCHUNK = NCT // CQ       # 4 chunks

# packed f32 constant-tile column offsets: eye | thr
EYE0, THR0 = 0, 128
CONST_W = 128 + HL
# f16 mask tile: 128-wide causal triangle then a 128-wide ones block
TRI0, ONES0 = 0, 128
MSKS_W = 256


def build():
    nc = bacc.Bacc("TRN2", target_bir_lowering=False, debug=False)

    xT = nc.dram_tensor("xT", [C, T], F32R, kind="ExternalInput").ap()
    wq = nc.dram_tensor("wq", [C, DL], F32R, kind="ExternalInput").ap()
    wk = nc.dram_tensor("wk", [C, D], F32R, kind="ExternalInput").ap()
    wv = nc.dram_tensor("wv", [C, D], F32R, kind="ExternalInput").ap()
    wo = nc.dram_tensor("wo", [DL, C], F16, kind="ExternalInput").ap()
    msks = nc.dram_tensor("msks", [128, MSKS_W], F16, kind="ExternalInput").ap()
    cs = nc.dram_tensor("cs", [R, T], F32, kind="ExternalInput").ap()
    sn = nc.dram_tensor("sn", [R, T], F32, kind="ExternalInput").ap()
    cst = nc.dram_tensor("cst", [128, CONST_W], F32, kind="ExternalInput").ap()
    ypT = nc.dram_tensor("ypT", [C, T], F16, kind="ExternalOutput").ap()

    with tile.TileContext(nc) as tc:
        with (
            tc.tile_pool(name="persist", bufs=1) as persist,
            tc.tile_pool(name="wpool", bufs=1) as wpool,
            tc.tile_pool(name="xpool", bufs=3) as xpool,
            tc.tile_pool(name="espool", bufs=2) as espool,
            tc.tile_pool(name="blk", bufs=2) as blk,
            tc.tile_pool(name="small", bufs=2) as small,
            tc.tile_pool(name="stgp", bufs=4) as stgp,
            tc.tile_pool(name="psum", bufs=1, space="PSUM") as psum,
        ):
            # ---- persistent SBUF ----
            kt = persist.tile([128, T], F32R)    # k^T (D x T), rope applied
            vn = persist.tile([128, T], F16)     # v natural; s-tile i at cols [128i,128i+128)
            cs_sb = persist.tile([R, T], F32)    # cos^T
            sn_sb = persist.tile([R, T], F32)    # sign-fixed sin^T: [-sinT[0:32] ; sinT[32:64]]
            msks_sb = persist.tile([128, MSKS_W], F16)
            cst_sb = persist.tile([128, CONST_W], F32)
            nc.sync.dma_start(msks_sb[:], msks)
            nc.sync.dma_start(cst_sb[:], cst)
            nc.sync.dma_start(cs_sb[:], cs)
            nc.sync.dma_start(sn_sb[:], sn)
            eye_sb = cst_sb[:, EYE0 : EYE0 + 128]
            thr_sb = cst_sb[:, THR0 : THR0 + HL]
            tri_sb = msks_sb[:, TRI0 : TRI0 + 128]
            ones_sb = msks_sb[:, ONES0 : ONES0 + 128]

            # ---- weights, interleaved with block-0 x chunks so the first
            # projection matmuls start as soon as chunk 0 + its weights land
            wq_sb = wpool.tile([128, NCT, DL], F32R, tag="wq", name="wq_sb")
            wk_sb = wpool.tile([128, NCT, D], F32R, tag="wk", name="wk_sb")
            wv_sb = wpool.tile([128, NCT, D], F32R, tag="wv", name="wv_sb")
            wo_sb = wpool.tile([128, HL, C], F16, tag="wo", name="wo_sb")
            xs0_chunks = []
            for ch in range(NCHUNK):
                xs = xpool.tile([128, CQ, TB], F32R, tag="xs", name=f"xs_0_{ch}")
                for ci in range(CQ):
                    c = ch * CQ + ci
                    csl = slice(128 * c, 128 * (c + 1))
                    nc.sync.dma_start(xs[:, ci, :], xT[csl, 0:TB])
                    nc.sync.dma_start(wq_sb[:, c, :], wq[csl, :])
                    nc.sync.dma_start(wk_sb[:, c, :], wk[csl, :])
                    nc.sync.dma_start(wv_sb[:, c, :], wv[csl, :])
                xs0_chunks.append(xs)

            def rope(th, dcols, tcols):
                """In-place partial RoPE on rows 0:R of region th[:, dcols].

                rotate-half via two partition-shifted single-input copies
                (legal on ACT), then partition-aligned tensor_tensor ops:
                  out[0:64] = q[0:64]*cos + rot*sin_signed
                with rot = [q[32:64]; q[0:32]], sin_signed = [-sin_lo; sin_hi].
                """
                hw = R // 2  # 32
                rot = small.tile([R, HL * TB], F32R, tag="ropeq", bufs=1, name="rope_rot")
                nc.scalar.copy(rot[0:hw, 0:TB], th[hw:R, dcols])
                nc.scalar.copy(rot[hw:R, 0:TB], th[0:hw, dcols])
                nc.vector.tensor_tensor(th[0:R, dcols], th[0:R, dcols], cs_sb[:, tcols], op=AluOpType.mult)
                nc.vector.tensor_tensor(rot[:, 0:TB], rot[:, 0:TB], sn_sb[:, tcols], op=AluOpType.mult)
                nc.vector.tensor_tensor(th[0:R, dcols], th[0:R, dcols], rot[:, 0:TB], op=AluOpType.add)

            def rope_q(qtb, tcols):
                """Batched RoPE over all HL head slices of qtb (same t-range),
                broadcasting cos/sin across the head dim with stride-0 APs."""
                hw = R // 2
                W = HL * TB
                rot = small.tile([R, W], F32R, tag="ropeq", bufs=1, name="ropeq_rot")
                nc.scalar.copy(rot[0:hw, :], qtb[hw:R, :])
                nc.scalar.copy(rot[hw:R, :], qtb[0:hw, :])
                qv = qtb[0:R, :].rearrange("p (r n) -> p r n", r=HL)
                rv = rot[:].rearrange("p (r n) -> p r n", r=HL)
                cb = cs_sb[:, tcols][:, None, :].broadcast_to([R, HL, TB])
                sb = sn_sb[:, tcols][:, None, :].broadcast_to([R, HL, TB])
                nc.vector.tensor_tensor(qv, qv, cb, op=AluOpType.mult)
                nc.vector.tensor_tensor(rv, rv, sb, op=AluOpType.mult)
                nc.vector.tensor_tensor(qv, qv, rv, op=AluOpType.add)

            qtbs = {}

            def proj_block(j):
                """Projections, RoPE, and v-transpose for t-block j."""
                tsl = slice(j * TB, (j + 1) * TB)
                if j == 0:
                    xs_chunks = xs0_chunks
                else:
                    xs_chunks = []
                    for ch in range(NCHUNK):
                        xs = xpool.tile([128, CQ, TB], F32R, tag="xs", name=f"xs_{j}_{ch}")
                        for ci in range(CQ):
                            c = ch * CQ + ci
                            nc.sync.dma_start(xs[:, ci, :], xT[128 * c : 128 * (c + 1), tsl])
                        xs_chunks.append(xs)

                # All 6 projection accumulators open at once; consume each
                # xs chunk fully before the next (xpool bufs=2 then suffices).
                qtb = blk.tile([128, HL * TB], F32R, tag="qtb", name=f"qtb_{j}")
                qtbs[j] = qtb
                qps = [
                    psum.tile([128, TB], F32, tag="a", bufs=4, name=f"qp_{j}_{h}")
                    for h in range(HL)
                ]
                kp = psum.tile([128, TB], F32, tag="b", bufs=2, name=f"kp_{j}")
                vp = psum.tile([128, TB], F32, tag="b", bufs=2, name=f"vp_{j}")
                groups = [(qps[h], wq_sb, 128 * h, 128) for h in range(HL)]
                groups += [(kp, wk_sb, 0, D), (vp, wv_sb, 0, D)]
                for ch in range(NCHUNK):
                    for gp, w_sb, col0, ncols in groups:
                        for ci in range(CQ):
                            c = ch * CQ + ci
                            nc.tensor.matmul(
                                gp[:],
                                w_sb[:, c, col0 : col0 + ncols],
                                xs_chunks[ch][:, ci, :],
                                start=(c == 0),
                                stop=(c == NCT - 1),
                            )
                # Eviction copies and the v-transpose go FIRST: ACT is strict
                # FIFO, and the att_{j-1} score tiles (PSUM tag "c") wait on
                # the transposes here — queueing the 2x2us rope copies ahead
                # of them would stall the PE at every block boundary.
                for h in range(HL):
                    nc.scalar.copy(qtb[:, TB * h : TB * (h + 1)], qps[h][:])
                nc.scalar.copy(kt[:, tsl], kp[:])
                vt_tmp = small.tile([128, TB], F32, tag="vt", bufs=1, name=f"vt_{j}")
                nc.scalar.copy(vt_tmp[:], vp[:])
                for u in range(TB // 128):
                    tp = psum.tile([128, 128], F32, tag="c", bufs=2, name=f"tp_{j}_{u}")
                    nc.tensor.transpose(tp[:], vt_tmp[:, 128 * u : 128 * (u + 1)], eye_sb)
                    s_idx = j * (TB // 128) + u
                    nc.vector.tensor_copy(vn[:, 128 * s_idx : 128 * (s_idx + 1)], tp[:])
                rope(kt, tsl, tsl)
                rope_q(qtb, tsl)

                if j == 0:
                    # wo is first needed by block 0's output projection;
                    # loading it here overlaps the DMA with compute.
                    for d in range(HL):
                        nc.sync.dma_start(wo_sb[:, d, :], wo[128 * d : 128 * (d + 1), :])

            def att_block(j):
                """Attention + output projection for t-block j (all local heads)."""
                tsl = slice(j * TB, (j + 1) * TB)
                qtb = qtbs.pop(j)
                nst = 4 * j + 4  # causal: s-tiles 0 .. 4j+3
                ytb = blk.tile([128, HL * TB], F16, tag="ytb", name=f"ytb_{j}")
                for h in range(HL):
                    qsl = slice(TB * h, TB * (h + 1))
                    esb = espool.tile([128, nst * TB], F16, tag="es", name=f"es_{j}_{h}")
                    # offsets: diagonal s-tile dpos only reaches t >= 128*dpos
                    offs = [max(0, (i - 4 * j) * 128) for i in range(nst)]
                    # phase A: scores + exp, restricted to the causally
                    # reachable range; tri-mask on the 128-wide diagonal block
                    for i in range(nst):
                        off = offs[i]
                        ssl = slice(128 * i, 128 * (i + 1))
                        sp = psum.tile([128, TB], F32, tag="c", bufs=2, name=f"sp_{j}_{h}_{i}")
                        nc.tensor.matmul(
                            sp[:, off:TB],
                            kt[:, ssl],
                            qtb[:, TB * h + off : TB * (h + 1)],
                            start=True,
                            stop=True,
                        )
                        es = esb[:, TB * i + off : TB * (i + 1)]
                        nc.scalar.activation(es, sp[:, off:TB], EXP, scale=SCALE)
                        if i - 4 * j >= 0:
                            nc.vector.tensor_tensor(
                                esb[:, TB * i + off : TB * i + off + 128],
                                esb[:, TB * i + off : TB * i + off + 128],
                                tri_sb,
                                op=AluOpType.mult,
                            )
                    # phase B: denominator (dense PE accumulation, f16)
                    dn = psum.tile([128, TB], F32, tag="b", bufs=2, name=f"dn_{j}_{h}")
                    for i in range(nst):
                        off = offs[i]
                        nc.tensor.matmul(
                            dn[:, off:TB], ones_sb, esb[:, TB * i + off : TB * (i + 1)],
                            start=(i == 0), stop=(i == nst - 1),
                        )
                    # phase C: threshold row (f16) and 1/denom (fast NR reciprocal)
                    work = small.tile([128, TB], F32, tag="work", bufs=2, name=f"work_{j}_{h}")
                    cwork = small.tile([128, TB], F16, tag="cwork", bufs=2, name=f"cwork_{j}_{h}")
                    cthr = cwork[:]
                    rden = work[:]
                    nc.vector.tensor_scalar_mul(cthr, dn[:], thr_sb[:, h : h + 1])
                    nc.vector.reciprocal_approx_fast(out=rden, in_=dn[:])
                    # phase D: gating es *= (es >= cthr); one big op over the
                    # full-width tiles, per-tile ops over the diagonal ones
                    # (their dead ranges are never written, so never read).
                    # sized for the largest single gating op (12 full tiles at
                    # j=3); per-op ranges index it from 0
                    msk = small.tile([128, 12 * TB], F16, tag="msk", bufs=1, name=f"msk_{j}_{h}")
                    # chunks of <=4 full tiles (so att@v can start before the
                    # whole head is gated), then per-tile diagonal ranges
                    gate_ranges = []
                    for i0 in range(0, 4 * j, 4):
                        gate_ranges.append((i0, i0 + 4))
                    for i in range(4 * j, nst):
                        gate_ranges.append((i, i + 1))
                    for i0, i1 in gate_ranges:
                        off = offs[i0]
                        gn = i1 - i0
                        if gn > 1:
                            ev = esb[:, TB * i0 : TB * i1].rearrange(
                                "p (r n) -> p r n", r=gn
                            )
                            mv = msk[:, 0 : TB * gn].rearrange(
                                "p (r n) -> p r n", r=gn
                            )
                            cb = cthr[:, None, :].broadcast_to([128, gn, TB])
                        else:
                            ev = esb[:, TB * i0 + off : TB * i1]
                            mv = msk[:, 0 : TB - off]
                            cb = cthr[:, off:TB]
                        nc.vector.tensor_tensor(mv, ev, cb, op=AluOpType.is_ge)
                        nc.vector.tensor_tensor(ev, ev, mv, op=AluOpType.mult)
                    # phase E: att @ v (dense, f16), then normalize
                    yp = psum.tile([128, TB], F32, tag="b", bufs=2, name=f"yp_{j}_{h}")
                    for i in range(nst):
                        off = offs[i]
                        nc.tensor.matmul(
                            yp[:, off:TB],
                            vn[:, 128 * i : 128 * (i + 1)],
                            esb[:, TB * i + off : TB * (i + 1)],
                            start=(i == 0), stop=(i == nst - 1),
                        )
                    nc.vector.tensor_tensor(ytb[:, qsl], yp[:], rden, op=AluOpType.mult)

                # --- output projection for block j (f16) ---
                # co-groups of 4 with head-major accumulation: head h's MMs
                # for the group run as soon as ytb_h is normalized, instead
                # of the whole projection waiting for the last head.
                for cg in range(0, C // 128, 4):
                    ops = [
                        psum.tile([128, TB], F32, tag="a", bufs=4, name=f"op_{j}_{cg + u}")
                        for u in range(4)
                    ]
                    for d in range(HL):
                        for u in range(4):
                            co = cg + u
                            nc.tensor.matmul(
                                ops[u][:],
                                wo_sb[:, d, 128 * co : 128 * (co + 1)],
                                ytb[:, TB * d : TB * (d + 1)],
                                start=(d == 0),
                                stop=(d == HL - 1),
                            )
                    for u in range(4):
                        co = cg + u
                        stg = stgp.tile([128, TB], F16, tag="stg", name=f"stg_{j}_{co}")
                        if co % 2 == 0:
                            nc.scalar.copy(stg[:], ops[u][:])
                        else:
                            nc.vector.tensor_copy(stg[:], ops[u][:])
                        nc.sync.dma_start(ypT[128 * co : 128 * (co + 1), tsl], stg[:])

            # ---- main loop: one-block software pipeline ----
            for j in range(NTB):
                proj_block(j)
                if j > 0:
                    att_block(j - 1)
            att_block(NTB - 1)

    nc.compile()
    return nc


_NC_CACHE = None


def _get_nc():
    global _NC_CACHE
    if _NC_CACHE is None:
        _NC_CACHE = build()
    return _NC_CACHE


def make_in_maps(x, cos, sin, Wq, Wk, Wv, Wo, gate):
    x = np.asarray(x, np.float32)
    cos = np.asarray(cos, np.float32)
    sin = np.asarray(sin, np.float32)
    Wq = np.asarray(Wq, np.float32)
    Wk = np.asarray(Wk, np.float32)
    Wv = np.asarray(Wv, np.float32)
    Wo = np.asarray(Wo, np.float32)
    gate = np.asarray(gate, np.float32)

    hw = R // 2
    cosT = np.ascontiguousarray(cos.T)  # (R, T)
    sinT = sin.T
    sn_signed = np.ascontiguousarray(np.concatenate([-sinT[0:hw], sinT[hw:R]], axis=0))
    thr_full = 1.0 / (1.0 + np.exp(-gate))  # sigmoid, (H,)
    cst_base = np.zeros((128, CONST_W), np.float32)
    cst_base[:, EYE0 : EYE0 + 128] = np.eye(128, dtype=np.float32)
    # f16 masks: 128-wide causal triangle (valid: s <= t) and a ones block
    msks = np.zeros((128, MSKS_W), np.float16)
    msks[:, TRI0 : TRI0 + 128] = np.triu(np.ones((128, 128), np.float32))
    msks[:, ONES0 : ONES0 + 128] = 1.0

    in_maps = []
    for core in range(NCORE):
        b, g = divmod(core, G)
        cst = cst_base.copy()
        cst[:, THR0 : THR0 + HL] = thr_full[HL * g : HL * (g + 1)]
        in_maps.append(
            {
                "xT": np.ascontiguousarray(x[b].T),
                "wq": np.ascontiguousarray(Wq[:, DL * g : DL * (g + 1)]),
                "wk": np.ascontiguousarray(Wk[:, D * g : D * (g + 1)]),
                "wv": np.ascontiguousarray(Wv[:, D * g : D * (g + 1)]),
                "wo": np.ascontiguousarray(Wo[DL * g : DL * (g + 1), :].astype(np.float16)),
                "msks": msks,
                "cs": cosT,
                "sn": sn_signed,
                "cst": cst,
            }
        )
    return in_maps


def run(inputs, trace=False, **kw):
    """Run on 8 NeuronCores; returns (y_full, BassKernelResults)."""
    nc = _get_nc()
    in_maps = make_in_maps(**inputs)
    res = run_bass_kernel_spmd(nc, in_maps, core_ids=list(range(NCORE)), trace=trace, **kw)
    y = np.zeros((B, T, C), np.float32)
    for core in range(NCORE):
        b = core // G
        y[b] += res.results[core]["ypT"].T.astype(np.float32)
    return y, res


def kernel(**inputs) -> np.ndarray:
    y, _ = run(inputs)
    return y


# revision 11
# speedup vs baseline: 1.2457x; 1.0276x over previous
"""Trainium2 Bass kernel: gated causal self-attention (GQA + partial RoPE).

Reference computation (per batch):
    q,k,v = x@Wq, x@Wk, x@Wv  (heads split, partial RoPE on first R dims)
    att = softmax(causal(q k^T / sqrt(D)))
    att = att * (att >= sigmoid(gate))          # post-softmax threshold gate
    y = (att @ v) @ Wo

Sharding over 8 NeuronCores: core = 4*b + g where b in {0,1} is the batch
(data parallel) and g in {0..3} is the KV-head group (tensor parallel:
Wq/Wk/Wv column-sharded, Wo row-sharded; gate sharded with heads).  Each
core computes a partial y^T (C x T) in f16; the host sums the 4 group
partials per batch (upcast to f32) and transposes.  The TxT score tensor
never leaves a core.

On-chip layout: everything is computed transposed (qT/kT are (D,T),
scores are S^T = (s,t)) so that
  - softmax denominator = ones-matmul accumulation (and it lands
    partition-broadcast, exactly what the gate compare needs),
  - att@v needs no transposes: out^T accumulates with v-natural tiles as
    the stationary operand and gated exp(S^T) moving,
  - the output projection consumes out^T directly and emits y^T.

Schedule: one-block software pipeline.  Per block j we emit the
projections + RoPE of block j and then the attention + output projection
of block j-1, so the RoPE chain (ACT/DVE) and the gating chain (DVE)
always have dense PE work (next block's projections / previous block's
attention) to hide under.  This also keeps the PE free of >3.4us idle
gaps, which would re-throttle its clock (HAM) to half rate.

PSUM (8 banks): tag "a" bufs=4 hosts the 4 q-head projection
accumulators and, later in priority order, the output-projection tiles;
tag "b" bufs=2 hosts k/v accumulators and the denominator / att@v
accumulators; tag "c" bufs=2 hosts score tiles and the v-transpose.

Diagonal (causal-boundary) score tiles only compute/exp the causally
reachable column range; the denominator and att@v matmuls restrict their
moving operands to the same range, so no masked-out work hits the PE.

Precision split: the threshold-sensitive path (x, Wq, Wk, q^T, k^T, S^T)
runs float32r (FP22 multiply, FP32 accumulate, full PE rate); the
post-exp path (exp tiles, ones, v, Wo, out^T) runs float16.  The y^T
partials are emitted in f16 (host accumulates in f32).
"""

import numpy as np

import concourse.bass as bass
import concourse.tile as tile
from concourse import bacc, mybir
from concourse.alu_op_type import AluOpType
from concourse.bass_utils import run_bass_kernel_spmd

# Problem shapes (hardcoded per contract)
B, T, C = 2, 2048, 2048
H, HKV, D = 16, 4, 128
R = 64
NCORE = 8
G = 4            # tensor-parallel degree over KV heads
HL = H // G      # 4 local q heads per core
DL = HL * D      # 512 local q dims per core
SCALE = float(D) ** -0.5

F32 = mybir.dt.float32
F32R = mybir.dt.float32r
F16 = mybir.dt.float16
EXP = mybir.ActivationFunctionType.Exp

TB = 512                 # t-block width
NTB = T // TB            # 4
NCT = C // 128           # 16 contraction tiles
CQ = 4                   # c-tiles per xs chunk
NCHUNK = NCT // CQ       # 4 chunks

# packed f32 constant-tile column offsets: eye | thr
EYE0, THR0 = 0, 128
CONST_W = 128 + HL
# f16 mask tile: 128-wide causal triangle then a 128-wide ones block
TRI0, ONES0 = 0, 128
MSKS_W = 256


def build():
    nc = bacc.Bacc("TRN2", target_bir_lowering=False, debug=False)

    xT = nc.dram_tensor("xT", [C, T], F32R, kind="ExternalInput").ap()
    wq = nc.dram_tensor("wq", [C, DL], F32R, kind="ExternalInput").ap()
    wk = nc.dram_tensor("wk", [C, D], F32R, kind="ExternalInput").ap()
    wv = nc.dram_tensor("wv", [C, D], F32R, kind="ExternalInput").ap()
    wo = nc.dram_tensor("wo", [DL, C], F16, kind="ExternalInput").ap()
    msks = nc.dram_tensor("msks", [128, MSKS_W], F16, kind="ExternalInput").ap()
    cs = nc.dram_tensor("cs", [R, T], F32, kind="ExternalInput").ap()
    sn = nc.dram_tensor("sn", [R, T], F32, kind="ExternalInput").ap()
    cst = nc.dram_tensor("cst", [128, CONST_W], F32, kind="ExternalInput").ap()
    ypT = nc.dram_tensor("ypT", [C, T], F16, kind="ExternalOutput").ap()

    with tile.TileContext(nc) as tc:
        with (
            tc.tile_pool(name="persist", bufs=1) as persist,
            tc.tile_pool(name="wpool", bufs=1) as wpool,
            tc.tile_pool(name="xpool", bufs=3) as xpool,
            tc.tile_pool(name="espool", bufs=2) as espool,
            tc.tile_pool(name="blk", bufs=2) as blk,
            tc.tile_pool(name="small", bufs=2) as small,
            tc.tile_pool(name="stgp", bufs=4) as stgp,
            tc.tile_pool(name="psum", bufs=1, space="PSUM") as psum,
        ):
            # ---- persistent SBUF ----
            kt = persist.tile([128, T], F32R)    # k^T (D x T), rope applied
            vn = persist.tile([128, T], F16)     # v natural; s-tile i at cols [128i,128i+128)
            cs_sb = persist.tile([R, T], F32)    # cos^T
            sn_sb = persist.tile([R, T], F32)    # sign-fixed sin^T: [-sinT[0:32] ; sinT[32:64]]
            msks_sb = persist.tile([128, MSKS_W], F16)
            cst_sb = persist.tile([128, CONST_W], F32)
            nc.sync.dma_start(msks_sb[:], msks)
            nc.sync.dma_start(cst_sb[:], cst)
            nc.sync.dma_start(cs_sb[:], cs)
            nc.sync.dma_start(sn_sb[:], sn)
            eye_sb = cst_sb[:, EYE0 : EYE0 + 128]
            thr_sb = cst_sb[:, THR0 : THR0 + HL]
            tri_sb = msks_sb[:, TRI0 : TRI0 + 128]
            ones_sb = msks_sb[:, ONES0 : ONES0 + 128]

            # ---- weights, interleaved with block-0 x chunks so the first
            # projection matmuls start as soon as chunk 0 + its weights land
            wq_sb = wpool.tile([128, NCT, DL], F32R, tag="wq", name="wq_sb")
            wk_sb = wpool.tile([128, NCT, D], F32R, tag="wk", name="wk_sb")
            wv_sb = wpool.tile([128, NCT, D], F32R, tag="wv", name="wv_sb")
            wo_sb = wpool.tile([128, HL, C], F16, tag="wo", name="wo_sb")
            xs0_chunks = []
            for ch in range(NCHUNK):
                xs = xpool.tile([128, CQ, TB], F32R, tag="xs", name=f"xs_0_{ch}")
                for ci in range(CQ):
                    c = ch * CQ + ci
                    csl = slice(128 * c, 128 * (c + 1))
                    nc.sync.dma_start(xs[:, ci, :], xT[csl, 0:TB])
                    nc.sync.dma_start(wq_sb[:, c, :], wq[csl, :])
                    nc.sync.dma_start(wk_sb[:, c, :], wk[csl, :])
                    nc.sync.dma_start(wv_sb[:, c, :], wv[csl, :])
                xs0_chunks.append(xs)

            def rope(th, dcols, tcols):
                """In-place partial RoPE on rows 0:R of region th[:, dcols].

                rotate-half via two partition-shifted single-input copies
                (legal on ACT), then partition-aligned tensor_tensor ops:
                  out[0:64] = q[0:64]*cos + rot*sin_signed
                with rot = [q[32:64]; q[0:32]], sin_signed = [-sin_lo; sin_hi].
                """
                hw = R // 2  # 32
                rot = small.tile([R, HL * TB], F32R, tag="ropeq", bufs=1, name="rope_rot")
                nc.scalar.copy(rot[0:hw, 0:TB], th[hw:R, dcols])
                nc.scalar.copy(rot[hw:R, 0:TB], th[0:hw, dcols])
                nc.vector.tensor_tensor(th[0:R, dcols], th[0:R, dcols], cs_sb[:, tcols], op=AluOpType.mult)
                nc.vector.tensor_tensor(rot[:, 0:TB], rot[:, 0:TB], sn_sb[:, tcols], op=AluOpType.mult)
                nc.vector.tensor_tensor(th[0:R, dcols], th[0:R, dcols], rot[:, 0:TB], op=AluOpType.add)

            def rope_q(qtb, tcols):
                """Batched RoPE over all HL head slices of qtb (same t-range),
                broadcasting cos/sin across the head dim with stride-0 APs."""
                hw = R // 2
                W = HL * TB
                rot = small.tile([R, W], F32R, tag="ropeq", bufs=1, name="ropeq_rot")
                nc.scalar.copy(rot[0:hw, :], qtb[hw:R, :])
                nc.scalar.copy(rot[hw:R, :], qtb[0:hw, :])
                qv = qtb[0:R, :].rearrange("p (r n) -> p r n", r=HL)
                rv = rot[:].rearrange("p (r n) -> p r n", r=HL)
                cb = cs_sb[:, tcols][:, None, :].broadcast_to([R, HL, TB])
                sb = sn_sb[:, tcols][:, None, :].broadcast_to([R, HL, TB])
                nc.vector.tensor_tensor(qv, qv, cb, op=AluOpType.mult)
                nc.vector.tensor_tensor(rv, rv, sb, op=AluOpType.mult)
                nc.vector.tensor_tensor(qv, qv, rv, op=AluOpType.add)

            qtbs = {}

            def proj_block(j):
                """Projections, RoPE, and v-transpose for t-block j."""
                tsl = slice(j * TB, (j + 1) * TB)
                if j == 0:
                    xs_chunks = xs0_chunks
                else:
                    xs_chunks = []
                    for ch in range(NCHUNK):
                        xs = xpool.tile([128, CQ, TB], F32R, tag="xs", name=f"xs_{j}_{ch}")
                        for ci in range(CQ):
                            c = ch * CQ + ci
                            nc.sync.dma_start(xs[:, ci, :], xT[128 * c : 128 * (c + 1), tsl])
                        xs_chunks.append(xs)

                # All 6 projection accumulators open at once; consume each
                # xs chunk fully before the next (xpool bufs=2 then suffices).
                qtb = blk.tile([128, HL * TB], F32R, tag="qtb", name=f"qtb_{j}")
                qtbs[j] = qtb
                qps = [
                    psum.tile([128, TB], F32, tag="a", bufs=4, name=f"qp_{j}_{h}")
                    for h in range(HL)
                ]
                kp = psum.tile([128, TB], F32, tag="b", bufs=2, name=f"kp_{j}")
                vp = psum.tile([128, TB], F32, tag="b", bufs=2, name=f"vp_{j}")
                groups = [(qps[h], wq_sb, 128 * h, 128) for h in range(HL)]
                groups += [(kp, wk_sb, 0, D), (vp, wv_sb, 0, D)]
                for ch in range(NCHUNK):
                    for gp, w_sb, col0, ncols in groups:
                        for ci in range(CQ):
                            c = ch * CQ + ci
                            nc.tensor.matmul(
                                gp[:],
                                w_sb[:, c, col0 : col0 + ncols],
                                xs_chunks[ch][:, ci, :],
                                start=(c == 0),
                                stop=(c == NCT - 1),
                            )
                # Eviction copies and the v-transpose go FIRST: ACT is strict
                # FIFO, and the att_{j-1} score tiles (PSUM tag "c") wait on
                # the transposes here — queueing the 2x2us rope copies ahead
                # of them would stall the PE at every block boundary.
                for h in range(HL):
                    nc.scalar.copy(qtb[:, TB * h : TB * (h + 1)], qps[h][:])
                nc.scalar.copy(kt[:, tsl], kp[:])
                vt_tmp = small.tile([128, TB], F32, tag="vt", bufs=1, name=f"vt_{j}")
                nc.scalar.copy(vt_tmp[:], vp[:])
                for u in range(TB // 128):
                    tp = psum.tile([128, 128], F32, tag="c", bufs=2, name=f"tp_{j}_{u}")
                    nc.tensor.transpose(tp[:], vt_tmp[:, 128 * u : 128 * (u + 1)], eye_sb)
                    s_idx = j * (TB // 128) + u
                    nc.vector.tensor_copy(vn[:, 128 * s_idx : 128 * (s_idx + 1)], tp[:])
                if j == 0:
                    rope(kt, tsl, tsl)
                    rope_q(qtb, tsl)

            def rope_block(j):
                """RoPE for block j.  Emitted AFTER att_block(j-1): rope is
                only needed by att_block(j), so its ACT copies / DVE ops must
                queue behind att_{j-1}'s exp and gating work (strict FIFO
                engines), not ahead of it."""
                tsl = slice(j * TB, (j + 1) * TB)
                rope(kt, tsl, tsl)
                rope_q(qtbs[j], tsl)

            def att_block(j):
                """Attention + output projection for t-block j (all local heads)."""
                tsl = slice(j * TB, (j + 1) * TB)
                qtb = qtbs.pop(j)
                if j == 0:
                    # wo is first needed by block 0's output projection;
                    # emitting the DMA here keeps it from competing with the
                    # higher-priority xs prefetch of block 1.
                    for d in range(HL):
                        nc.sync.dma_start(wo_sb[:, d, :], wo[128 * d : 128 * (d + 1), :])
                nst = 4 * j + 4  # causal: s-tiles 0 .. 4j+3
                ytb = blk.tile([128, HL * TB], F16, tag="ytb", name=f"ytb_{j}")
                for h in range(HL):
                    qsl = slice(TB * h, TB * (h + 1))
                    esb = espool.tile([128, nst * TB], F16, tag="es", name=f"es_{j}_{h}")
                    # offsets: diagonal s-tile dpos only reaches t >= 128*dpos
                    offs = [max(0, (i - 4 * j) * 128) for i in range(nst)]
                    # phase A: scores + exp, restricted to the causally
                    # reachable range; tri-mask on the 128-wide diagonal block
                    for i in range(nst):
                        off = offs[i]
                        ssl = slice(128 * i, 128 * (i + 1))
                        sp = psum.tile([128, TB], F32, tag="c", bufs=2, name=f"sp_{j}_{h}_{i}")
                        nc.tensor.matmul(
                            sp[:, off:TB],
                            kt[:, ssl],
                            qtb[:, TB * h + off : TB * (h + 1)],
                            start=True,
                            stop=True,
                        )
                        es = esb[:, TB * i + off : TB * (i + 1)]
                        nc.scalar.activation(es, sp[:, off:TB], EXP, scale=SCALE)
                        if i - 4 * j >= 0:
                            nc.vector.tensor_tensor(
                                esb[:, TB * i + off : TB * i + off + 128],
                                esb[:, TB * i + off : TB * i + off + 128],
                                tri_sb,
                                op=AluOpType.mult,
                            )
                    # phase B: denominator (dense PE accumulation, f16)
                    dn = psum.tile([128, TB], F32, tag="b", bufs=2, name=f"dn_{j}_{h}")
                    for i in range(nst):
                        off = offs[i]
                        nc.tensor.matmul(
                            dn[:, off:TB], ones_sb, esb[:, TB * i + off : TB * (i + 1)],
                            start=(i == 0), stop=(i == nst - 1),
                        )
                    # phase C: threshold row (f16) and 1/denom (fast NR reciprocal)
                    work = small.tile([128, TB], F32, tag="work", bufs=2, name=f"work_{j}_{h}")
                    cwork = small.tile([128, TB], F16, tag="cwork", bufs=2, name=f"cwork_{j}_{h}")
                    cthr = cwork[:]
                    rden = work[:]
                    nc.vector.tensor_scalar_mul(cthr, dn[:], thr_sb[:, h : h + 1])
                    nc.vector.reciprocal_approx_fast(out=rden, in_=dn[:])
                    # phase D: gating es *= (es >= cthr); one big op over the
                    # full-width tiles, per-tile ops over the diagonal ones
                    # (their dead ranges are never written, so never read).
                    # sized for the largest single gating op (12 full tiles at
                    # j=3); per-op ranges index it from 0
                    msk = small.tile([128, 12 * TB], F16, tag="msk", bufs=1, name=f"msk_{j}_{h}")
                    # chunks of <=4 full tiles (so att@v can start before the
                    # whole head is gated), then per-tile diagonal ranges
                    gate_ranges = []
                    for i0 in range(0, 4 * j, 4):
                        gate_ranges.append((i0, i0 + 4))
                    for i in range(4 * j, nst):
                        gate_ranges.append((i, i + 1))
                    for i0, i1 in gate_ranges:
                        off = offs[i0]
                        gn = i1 - i0
                        if gn > 1:
                            ev = esb[:, TB * i0 : TB * i1].rearrange(
                                "p (r n) -> p r n", r=gn
                            )
                            mv = msk[:, 0 : TB * gn].rearrange(
                                "p (r n) -> p r n", r=gn
                            )
                            cb = cthr[:, None, :].broadcast_to([128, gn, TB])
                        else:
                            ev = esb[:, TB * i0 + off : TB * i1]
                            mv = msk[:, 0 : TB - off]
                            cb = cthr[:, off:TB]
                        nc.vector.tensor_tensor(mv, ev, cb, op=AluOpType.is_ge)
                        nc.vector.tensor_tensor(ev, ev, mv, op=AluOpType.mult)
                    # phase E: att @ v (dense, f16), then normalize
                    yp = psum.tile([128, TB], F32, tag="b", bufs=2, name=f"yp_{j}_{h}")
                    for i in range(nst):
                        off = offs[i]
                        nc.tensor.matmul(
                            yp[:, off:TB],
                            vn[:, 128 * i : 128 * (i + 1)],
                            esb[:, TB * i + off : TB * (i + 1)],
                            start=(i == 0), stop=(i == nst - 1),
                        )
                    nc.vector.tensor_tensor(ytb[:, qsl], yp[:], rden, op=AluOpType.mult)

                # --- output projection for block j (f16) ---
                # co-groups of 4 with head-major accumulation: head h's MMs
                # for the group run as soon as ytb_h is normalized, instead
                # of the whole projection waiting for the last head.
                for cg in range(0, C // 128, 4):
                    ops = [
                        psum.tile([128, TB], F32, tag="a", bufs=4, name=f"op_{j}_{cg + u}")
                        for u in range(4)
                    ]
                    for d in range(HL):
                        for u in range(4):
                            co = cg + u
                            nc.tensor.matmul(
                                ops[u][:],
                                wo_sb[:, d, 128 * co : 128 * (co + 1)],
                                ytb[:, TB * d : TB * (d + 1)],
                                start=(d == 0),
                                stop=(d == HL - 1),
                            )
                    for u in range(4):
                        co = cg + u
                        stg = stgp.tile([128, TB], F16, tag="stg", name=f"stg_{j}_{co}")
                        if co % 2 == 0:
                            nc.scalar.copy(stg[:], ops[u][:])
                        else:
                            nc.vector.tensor_copy(stg[:], ops[u][:])
                        nc.sync.dma_start(ypT[128 * co : 128 * (co + 1), tsl], stg[:])

            # ---- main loop: one-block software pipeline ----
            for j in range(NTB):
                proj_block(j)
                if j > 0:
                    att_block(j - 1)
                    rope_block(j)
            att_block(NTB - 1)

    nc.compile()
    return nc


_NC_CACHE = None


def _get_nc():
    global _NC_CACHE
    if _NC_CACHE is None:
        _NC_CACHE = build()
    return _NC_CACHE


def make_in_maps(x, cos, sin, Wq, Wk, Wv, Wo, gate):
    x = np.asarray(x, np.float32)
    cos = np.asarray(cos, np.float32)
    sin = np.asarray(sin, np.float32)
    Wq = np.asarray(Wq, np.float32)
    Wk = np.asarray(Wk, np.float32)
    Wv = np.asarray(Wv, np.float32)
    Wo = np.asarray(Wo, np.float32)
    gate = np.asarray(gate, np.float32)

    hw = R // 2
    cosT = np.ascontiguousarray(cos.T)  # (R, T)
    sinT = sin.T
    sn_signed = np.ascontiguousarray(np.concatenate([-sinT[0:hw], sinT[hw:R]], axis=0))
    thr_full = 1.0 / (1.0 + np.exp(-gate))  # sigmoid, (H,)
    cst_base = np.zeros((128, CONST_W), np.float32)
    cst_base[:, EYE0 : EYE0 + 128] = np.eye(128, dtype=np.float32)
    # f16 masks: 128-wide causal triangle (valid: s <= t) and a ones block
    msks = np.zeros((128, MSKS_W), np.float16)
    msks[:, TRI0 : TRI0 + 128] = np.triu(np.ones((128, 128), np.float32))
    msks[:, ONES0 : ONES0 + 128] = 1.0

    in_maps = []
    for core in range(NCORE):
        b, g = divmod(core, G)
        cst = cst_base.copy()
        cst[:, THR0 : THR0 + HL] = thr_full[HL * g : HL * (g + 1)]
        in_maps.append(
            {
                "xT": np.ascontiguousarray(x[b].T),
                "wq": np.ascontiguousarray(Wq[:, DL * g : DL * (g + 1)]),
                "wk": np.ascontiguousarray(Wk[:, D * g : D * (g + 1)]),
                "wv": np.ascontiguousarray(Wv[:, D * g : D * (g + 1)]),
                "wo": np.ascontiguousarray(Wo[DL * g : DL * (g + 1), :].astype(np.float16)),
                "msks": msks,
                "cs": cosT,
                "sn": sn_signed,
                "cst": cst,
            }
        )
    return in_maps


def run(inputs, trace=False, **kw):
    """Run on 8 NeuronCores; returns (y_full, BassKernelResults)."""
    nc = _get_nc()
    in_maps = make_in_maps(**inputs)
    res = run_bass_kernel_spmd(nc, in_maps, core_ids=list(range(NCORE)), trace=trace, **kw)
    y = np.zeros((B, T, C), np.float32)
    for core in range(NCORE):
        b = core // G
        y[b] += res.results[core]["ypT"].T.astype(np.float32)
    return y, res


def kernel(**inputs) -> np.ndarray:
    y, _ = run(inputs)
    return y


# revision 13
# speedup vs baseline: 1.2856x; 1.0321x over previous
"""Trainium2 Bass kernel: gated causal self-attention (GQA + partial RoPE).

Reference computation (per batch):
    q,k,v = x@Wq, x@Wk, x@Wv  (heads split, partial RoPE on first R dims)
    att = softmax(causal(q k^T / sqrt(D)))
    att = att * (att >= sigmoid(gate))          # post-softmax threshold gate
    y = (att @ v) @ Wo

Sharding over 8 NeuronCores: core = 4*b + g where b in {0,1} is the batch
(data parallel) and g in {0..3} is the KV-head group (tensor parallel:
Wq/Wk/Wv column-sharded, Wo row-sharded; gate sharded with heads).  Each
core computes a partial y^T (C x T) in f16; the host sums the 4 group
partials per batch (upcast to f32) and transposes.  The TxT score tensor
never leaves a core.

On-chip layout: everything is computed transposed (qT/kT are (D,T),
scores are S^T = (s,t)) so that
  - softmax denominator = ones-matmul accumulation (and it lands
    partition-broadcast, exactly what the gate compare needs),
  - att@v needs no transposes: out^T accumulates with v-natural tiles as
    the stationary operand and gated exp(S^T) moving,
  - the output projection consumes out^T directly and emits y^T.

Schedule: one-block software pipeline.  Per block j we emit the
projections + RoPE of block j and then the attention + output projection
of block j-1, so the RoPE chain (ACT/DVE) and the gating chain (DVE)
always have dense PE work (next block's projections / previous block's
attention) to hide under.  This also keeps the PE free of >3.4us idle
gaps, which would re-throttle its clock (HAM) to half rate.

PSUM (8 banks): tag "a" bufs=4 hosts the 4 q-head projection
accumulators and, later in priority order, the output-projection tiles;
tag "b" bufs=2 hosts k/v accumulators and the denominator / att@v
accumulators; tag "c" bufs=2 hosts score tiles and the v-transpose.

Diagonal (causal-boundary) score tiles only compute/exp the causally
reachable column range; the denominator and att@v matmuls restrict their
moving operands to the same range, so no masked-out work hits the PE.

Precision split: the threshold-sensitive path (x, Wq, Wk, q^T, k^T, S^T)
runs float32r (FP22 multiply, FP32 accumulate, full PE rate); the
post-exp path (exp tiles, ones, v, Wo, out^T) runs float16.  The y^T
partials are emitted in f16 (host accumulates in f32).
"""

import numpy as np

import concourse.bass as bass
import concourse.tile as tile
from concourse import bacc, mybir
from concourse.alu_op_type import AluOpType
from concourse.bass_utils import run_bass_kernel_spmd

# Problem shapes (hardcoded per contract)
B, T, C = 2, 2048, 2048
H, HKV, D = 16, 4, 128
R = 64
NCORE = 8
G = 4            # tensor-parallel degree over KV heads
HL = H // G      # 4 local q heads per core
DL = HL * D      # 512 local q dims per core
SCALE = float(D) ** -0.5

F32 = mybir.dt.float32
F32R = mybir.dt.float32r
F16 = mybir.dt.float16
EXP = mybir.ActivationFunctionType.Exp

TB = 512                 # t-block width
NTB = T // TB            # 4
NCT = C // 128           # 16 contraction tiles
CQ = 4                   # c-tiles per xs chunk
NCHUNK = NCT // CQ       # 4 chunks

# packed f32 constant-tile column offsets: eye | thr
EYE0, THR0 = 0, 128
CONST_W = 128 + HL
# f16 mask tile: 128-wide causal triangle then a 128-wide ones block
TRI0, ONES0 = 0, 128
MSKS_W = 256


def build():
    nc = bacc.Bacc("TRN2", target_bir_lowering=False, debug=False)

    xT = nc.dram_tensor("xT", [C, T], F32R, kind="ExternalInput").ap()
    wq = nc.dram_tensor("wq", [C, DL], F32R, kind="ExternalInput").ap()
    wk = nc.dram_tensor("wk", [C, D], F32R, kind="ExternalInput").ap()
    wv = nc.dram_tensor("wv", [C, D], F32R, kind="ExternalInput").ap()
    wo = nc.dram_tensor("wo", [DL, C], F16, kind="ExternalInput").ap()
    msks = nc.dram_tensor("msks", [128, MSKS_W], F16, kind="ExternalInput").ap()
    cs = nc.dram_tensor("cs", [R, T], F32, kind="ExternalInput").ap()
    sn = nc.dram_tensor("sn", [R, T], F32, kind="ExternalInput").ap()
    cst = nc.dram_tensor("cst", [128, CONST_W], F32, kind="ExternalInput").ap()
    ypT = nc.dram_tensor("ypT", [C, T], F16, kind="ExternalOutput").ap()

    with tile.TileContext(nc) as tc:
        with (
            tc.tile_pool(name="persist", bufs=1) as persist,
            tc.tile_pool(name="wpool", bufs=1) as wpool,
            tc.tile_pool(name="xpool", bufs=3) as xpool,
            tc.tile_pool(name="espool", bufs=2) as espool,
            tc.tile_pool(name="blk", bufs=2) as blk,
            tc.tile_pool(name="small", bufs=2) as small,
            tc.tile_pool(name="stgp", bufs=4) as stgp,
            tc.tile_pool(name="psum", bufs=1, space="PSUM") as psum,
        ):
            # ---- persistent SBUF ----
            kt = persist.tile([128, T], F32R)    # k^T (D x T), rope applied
            vn = persist.tile([128, T], F16)     # v natural; s-tile i at cols [128i,128i+128)
            cs_sb = persist.tile([R, T], F32)    # cos^T
            sn_sb = persist.tile([R, T], F32)    # sign-fixed sin^T: [-sinT[0:32] ; sinT[32:64]]
            msks_sb = persist.tile([128, MSKS_W], F16)
            cst_sb = persist.tile([128, CONST_W], F32)
            nc.sync.dma_start(msks_sb[:], msks)
            nc.sync.dma_start(cst_sb[:], cst)
            nc.sync.dma_start(cs_sb[:], cs)
            nc.sync.dma_start(sn_sb[:], sn)
            eye_sb = cst_sb[:, EYE0 : EYE0 + 128]
            thr_sb = cst_sb[:, THR0 : THR0 + HL]
            tri_sb = msks_sb[:, TRI0 : TRI0 + 128]
            ones_sb = msks_sb[:, ONES0 : ONES0 + 128]

            # ---- weights, interleaved with block-0 x chunks so the first
            # projection matmuls start as soon as chunk 0 + its weights land
            wq_sb = wpool.tile([128, NCT, DL], F32R, tag="wq", name="wq_sb")
            wk_sb = wpool.tile([128, NCT, D], F32R, tag="wk", name="wk_sb")
            wv_sb = wpool.tile([128, NCT, D], F32R, tag="wv", name="wv_sb")
            wo_sb = wpool.tile([128, HL, C], F16, tag="wo", name="wo_sb")
            xs0_chunks = []
            for ch in range(NCHUNK):
                xs = xpool.tile([128, CQ, TB], F32R, tag="xs", name=f"xs_0_{ch}")
                for ci in range(CQ):
                    c = ch * CQ + ci
                    csl = slice(128 * c, 128 * (c + 1))
                    nc.sync.dma_start(xs[:, ci, :], xT[csl, 0:TB])
                    nc.sync.dma_start(wq_sb[:, c, :], wq[csl, :])
                    nc.sync.dma_start(wk_sb[:, c, :], wk[csl, :])
                    nc.sync.dma_start(wv_sb[:, c, :], wv[csl, :])
                xs0_chunks.append(xs)

            def rope(th, dcols, tcols):
                """In-place partial RoPE on rows 0:R of region th[:, dcols].

                rotate-half via two partition-shifted single-input copies
                (legal on ACT), then partition-aligned tensor_tensor ops:
                  out[0:64] = q[0:64]*cos + rot*sin_signed
                with rot = [q[32:64]; q[0:32]], sin_signed = [-sin_lo; sin_hi].
                """
                hw = R // 2  # 32
                rot = small.tile([R, HL * TB], F32R, tag="ropeq", bufs=1, name="rope_rot")
                nc.scalar.copy(rot[0:hw, 0:TB], th[hw:R, dcols])
                nc.scalar.copy(rot[hw:R, 0:TB], th[0:hw, dcols])
                nc.vector.tensor_tensor(th[0:R, dcols], th[0:R, dcols], cs_sb[:, tcols], op=AluOpType.mult)
                nc.vector.tensor_tensor(rot[:, 0:TB], rot[:, 0:TB], sn_sb[:, tcols], op=AluOpType.mult)
                nc.vector.tensor_tensor(th[0:R, dcols], th[0:R, dcols], rot[:, 0:TB], op=AluOpType.add)

            def rope_q(qtb, tcols):
                """Batched RoPE over all HL head slices of qtb (same t-range),
                broadcasting cos/sin across the head dim with stride-0 APs."""
                hw = R // 2
                W = HL * TB
                rot = small.tile([R, W], F32R, tag="ropeq", bufs=1, name="ropeq_rot")
                nc.scalar.copy(rot[0:hw, :], qtb[hw:R, :])
                nc.scalar.copy(rot[hw:R, :], qtb[0:hw, :])
                qv = qtb[0:R, :].rearrange("p (r n) -> p r n", r=HL)
                rv = rot[:].rearrange("p (r n) -> p r n", r=HL)
                cb = cs_sb[:, tcols][:, None, :].broadcast_to([R, HL, TB])
                sb = sn_sb[:, tcols][:, None, :].broadcast_to([R, HL, TB])
                nc.vector.tensor_tensor(qv, qv, cb, op=AluOpType.mult)
                nc.vector.tensor_tensor(rv, rv, sb, op=AluOpType.mult)
                nc.vector.tensor_tensor(qv, qv, rv, op=AluOpType.add)

            qtbs = {}

            def proj_block(j):
                """Projections, RoPE, and v-transpose for t-block j."""
                tsl = slice(j * TB, (j + 1) * TB)
                if j == 0:
                    xs_chunks = xs0_chunks
                else:
                    xs_chunks = []
                    for ch in range(NCHUNK):
                        xs = xpool.tile([128, CQ, TB], F32R, tag="xs", name=f"xs_{j}_{ch}")
                        for ci in range(CQ):
                            c = ch * CQ + ci
                            nc.sync.dma_start(xs[:, ci, :], xT[128 * c : 128 * (c + 1), tsl])
                        xs_chunks.append(xs)

                # All 6 projection accumulators open at once; consume each
                # xs chunk fully before the next (xpool bufs=2 then suffices).
                qtb = blk.tile([128, HL * TB], F32R, tag="qtb", name=f"qtb_{j}")
                qtbs[j] = qtb
                qps = [
                    psum.tile([128, TB], F32, tag="a", bufs=4, name=f"qp_{j}_{h}")
                    for h in range(HL)
                ]
                kp = psum.tile([128, TB], F32, tag="b", bufs=2, name=f"kp_{j}")
                vp = psum.tile([128, TB], F32, tag="b", bufs=2, name=f"vp_{j}")
                # k/v first: their accumulators finish early in the last
                # chunk, so the kt/vt evictions and the v-transpose overlap
                # the remaining q matmuls instead of trailing them.
                groups = [(kp, wk_sb, 0, D), (vp, wv_sb, 0, D)]
                groups += [(qps[h], wq_sb, 128 * h, 128) for h in range(HL)]
                for ch in range(NCHUNK):
                    for gp, w_sb, col0, ncols in groups:
                        for ci in range(CQ):
                            c = ch * CQ + ci
                            nc.tensor.matmul(
                                gp[:],
                                w_sb[:, c, col0 : col0 + ncols],
                                xs_chunks[ch][:, ci, :],
                                start=(c == 0),
                                stop=(c == NCT - 1),
                            )
                # Evictions: kt/vt/vn on ACT (queued ahead of att_{j-1}'s
                # exps, but short), qtb on DVE (ACT must stay clear so the
                # exps that pace att_{j-1}'s score tiles start immediately).
                # The v-transpose uses tag "b" (freed by the k/v evictions)
                # so the score tiles in tag "c" are never blocked on it.
                nc.scalar.copy(kt[:, tsl], kp[:])
                vt_tmp = small.tile([128, TB], F32, tag="vt", bufs=1, name=f"vt_{j}")
                nc.scalar.copy(vt_tmp[:], vp[:])
                for u in range(TB // 128):
                    tp = psum.tile([128, 128], F32, tag="b", bufs=2, name=f"tp_{j}_{u}")
                    nc.tensor.transpose(tp[:], vt_tmp[:, 128 * u : 128 * (u + 1)], eye_sb)
                    s_idx = j * (TB // 128) + u
                    nc.scalar.copy(vn[:, 128 * s_idx : 128 * (s_idx + 1)], tp[:])
                for h in range(HL):
                    nc.vector.tensor_copy(qtb[:, TB * h : TB * (h + 1)], qps[h][:])
                if j == 0:
                    rope(kt, tsl, tsl)
                    rope_q(qtb, tsl)

            def rope_block(j):
                """RoPE for block j.  Emitted AFTER att_block(j-1): rope is
                only needed by att_block(j), so its ACT copies / DVE ops must
                queue behind att_{j-1}'s exp and gating work (strict FIFO
                engines), not ahead of it."""
                tsl = slice(j * TB, (j + 1) * TB)
                rope(kt, tsl, tsl)
                rope_q(qtbs[j], tsl)

            def att_block(j):
                """Attention + output projection for t-block j (all local heads)."""
                tsl = slice(j * TB, (j + 1) * TB)
                qtb = qtbs.pop(j)
                if j == 0:
                    # wo is first needed by block 0's output projection;
                    # emitting the DMA here keeps it from competing with the
                    # higher-priority xs prefetch of block 1.
                    for d in range(HL):
                        nc.sync.dma_start(wo_sb[:, d, :], wo[128 * d : 128 * (d + 1), :])
                nst = 4 * j + 4  # causal: s-tiles 0 .. 4j+3
                ytb = blk.tile([128, HL * TB], F16, tag="ytb", name=f"ytb_{j}")
                for h in range(HL):
                    qsl = slice(TB * h, TB * (h + 1))
                    esb = espool.tile([128, nst * TB], F16, tag="es", name=f"es_{j}_{h}")
                    # offsets: diagonal s-tile dpos only reaches t >= 128*dpos
                    offs = [max(0, (i - 4 * j) * 128) for i in range(nst)]
                    # phase A: scores + exp, restricted to the causally
                    # reachable range; tri-mask on the 128-wide diagonal block
                    for i in range(nst):
                        off = offs[i]
                        ssl = slice(128 * i, 128 * (i + 1))
                        sp = psum.tile([128, TB], F32, tag="c", bufs=2, name=f"sp_{j}_{h}_{i}")
                        nc.tensor.matmul(
                            sp[:, off:TB],
                            kt[:, ssl],
                            qtb[:, TB * h + off : TB * (h + 1)],
                            start=True,
                            stop=True,
                        )
                        es = esb[:, TB * i + off : TB * (i + 1)]
                        nc.scalar.activation(es, sp[:, off:TB], EXP, scale=SCALE)
                        if i - 4 * j >= 0:
                            nc.vector.tensor_tensor(
                                esb[:, TB * i + off : TB * i + off + 128],
                                esb[:, TB * i + off : TB * i + off + 128],
                                tri_sb,
                                op=AluOpType.mult,
                            )
                    # phase B: denominator (dense PE accumulation, f16)
                    dn = psum.tile([128, TB], F32, tag="b", bufs=2, name=f"dn_{j}_{h}")
                    for i in range(nst):
                        off = offs[i]
                        nc.tensor.matmul(
                            dn[:, off:TB], ones_sb, esb[:, TB * i + off : TB * (i + 1)],
                            start=(i == 0), stop=(i == nst - 1),
                        )
                    # phase C: threshold row (f16) and 1/denom (fast NR reciprocal)
                    work = small.tile([128, TB], F32, tag="work", bufs=2, name=f"work_{j}_{h}")
                    cwork = small.tile([128, TB], F16, tag="cwork", bufs=2, name=f"cwork_{j}_{h}")
                    cthr = cwork[:]
                    rden = work[:]
                    nc.vector.tensor_scalar_mul(cthr, dn[:], thr_sb[:, h : h + 1])
                    nc.vector.reciprocal_approx_fast(out=rden, in_=dn[:])
                    # phase D: gating es *= (es >= cthr); one big op over the
                    # full-width tiles, per-tile ops over the diagonal ones
                    # (their dead ranges are never written, so never read).
                    # sized for the largest single gating op (12 full tiles at
                    # j=3); per-op ranges index it from 0
                    msk = small.tile([128, 12 * TB], F16, tag="msk", bufs=1, name=f"msk_{j}_{h}")
                    # chunks of <=4 full tiles (so att@v can start before the
                    # whole head is gated), then per-tile diagonal ranges
                    gate_ranges = []
                    for i0 in range(0, 4 * j, 4):
                        gate_ranges.append((i0, i0 + 4))
                    for i in range(4 * j, nst):
                        gate_ranges.append((i, i + 1))
                    for i0, i1 in gate_ranges:
                        off = offs[i0]
                        gn = i1 - i0
                        if gn > 1:
                            ev = esb[:, TB * i0 : TB * i1].rearrange(
                                "p (r n) -> p r n", r=gn
                            )
                            mv = msk[:, 0 : TB * gn].rearrange(
                                "p (r n) -> p r n", r=gn
                            )
                            cb = cthr[:, None, :].broadcast_to([128, gn, TB])
                        else:
                            ev = esb[:, TB * i0 + off : TB * i1]
                            mv = msk[:, 0 : TB - off]
                            cb = cthr[:, off:TB]
                        nc.vector.tensor_tensor(mv, ev, cb, op=AluOpType.is_ge)
                        nc.vector.tensor_tensor(ev, ev, mv, op=AluOpType.mult)
                    # phase E: att @ v (dense, f16), then normalize
                    yp = psum.tile([128, TB], F32, tag="b", bufs=2, name=f"yp_{j}_{h}")
                    for i in range(nst):
                        off = offs[i]
                        nc.tensor.matmul(
                            yp[:, off:TB],
                            vn[:, 128 * i : 128 * (i + 1)],
                            esb[:, TB * i + off : TB * (i + 1)],
                            start=(i == 0), stop=(i == nst - 1),
                        )
                    nc.vector.tensor_tensor(ytb[:, qsl], yp[:], rden, op=AluOpType.mult)

                # --- output projection for block j (f16) ---
                # co-groups of 4 with head-major accumulation: head h's MMs
                # for the group run as soon as ytb_h is normalized, instead
                # of the whole projection waiting for the last head.
                for cg in range(0, C // 128, 4):
                    ops = [
                        psum.tile([128, TB], F32, tag="a", bufs=4, name=f"op_{j}_{cg + u}")
                        for u in range(4)
                    ]
                    for d in range(HL):
                        for u in range(4):
                            co = cg + u
                            nc.tensor.matmul(
                                ops[u][:],
                                wo_sb[:, d, 128 * co : 128 * (co + 1)],
                                ytb[:, TB * d : TB * (d + 1)],
                                start=(d == 0),
                                stop=(d == HL - 1),
                            )
                    for u in range(4):
                        co = cg + u
                        stg = stgp.tile([128, TB], F16, tag="stg", name=f"stg_{j}_{co}")
                        if co % 2 == 0:
                            nc.scalar.copy(stg[:], ops[u][:])
                        else:
                            nc.vector.tensor_copy(stg[:], ops[u][:])
                        nc.sync.dma_start(ypT[128 * co : 128 * (co + 1), tsl], stg[:])

            # ---- main loop: one-block software pipeline ----
            for j in range(NTB):
                proj_block(j)
                if j > 0:
                    att_block(j - 1)
                    rope_block(j)
            att_block(NTB - 1)

    nc.compile()
    return nc


_NC_CACHE = None


def _get_nc():
    global _NC_CACHE
    if _NC_CACHE is None:
        _NC_CACHE = build()
    return _NC_CACHE


def make_in_maps(x, cos, sin, Wq, Wk, Wv, Wo, gate):
    x = np.asarray(x, np.float32)
    cos = np.asarray(cos, np.float32)
    sin = np.asarray(sin, np.float32)
    Wq = np.asarray(Wq, np.float32)
    Wk = np.asarray(Wk, np.float32)
    Wv = np.asarray(Wv, np.float32)
    Wo = np.asarray(Wo, np.float32)
    gate = np.asarray(gate, np.float32)

    hw = R // 2
    cosT = np.ascontiguousarray(cos.T)  # (R, T)
    sinT = sin.T
    sn_signed = np.ascontiguousarray(np.concatenate([-sinT[0:hw], sinT[hw:R]], axis=0))
    thr_full = 1.0 / (1.0 + np.exp(-gate))  # sigmoid, (H,)
    cst_base = np.zeros((128, CONST_W), np.float32)
    cst_base[:, EYE0 : EYE0 + 128] = np.eye(128, dtype=np.float32)
    # f16 masks: 128-wide causal triangle (valid: s <= t) and a ones block
    msks = np.zeros((128, MSKS_W), np.float16)
    msks[:, TRI0 : TRI0 + 128] = np.triu(np.ones((128, 128), np.float32))
    msks[:, ONES0 : ONES0 + 128] = 1.0

    in_maps = []
    for core in range(NCORE):
        b, g = divmod(core, G)
        cst = cst_base.copy()
        cst[:, THR0 : THR0 + HL] = thr_full[HL * g : HL * (g + 1)]
        in_maps.append(
            {
                "xT": np.ascontiguousarray(x[b].T),
                "wq": np.ascontiguousarray(Wq[:, DL * g : DL * (g + 1)]),
                "wk": np.ascontiguousarray(Wk[:, D * g : D * (g + 1)]),
                "wv": np.ascontiguousarray(Wv[:, D * g : D * (g + 1)]),
                "wo": np.ascontiguousarray(Wo[DL * g : DL * (g + 1), :].astype(np.float16)),
                "msks": msks,
                "cs": cosT,
                "sn": sn_signed,
                "cst": cst,
            }
        )
    return in_maps


def run(inputs, trace=False, **kw):
    """Run on 8 NeuronCores; returns (y_full, BassKernelResults)."""
    nc = _get_nc()
    in_maps = make_in_maps(**inputs)
    res = run_bass_kernel_spmd(nc, in_maps, core_ids=list(range(NCORE)), trace=trace, **kw)
    y = np.zeros((B, T, C), np.float32)
    for core in range(NCORE):
        b = core // G
        y[b] += res.results[core]["ypT"].T.astype(np.float32)
    return y, res


def kernel(**inputs) -> np.ndarray:
    y, _ = run(inputs)
    return y


# revision 15
# speedup vs baseline: 1.3010x; 1.0119x over previous
"""Trainium2 Bass kernel: gated causal self-attention (GQA + partial RoPE).

Reference computation (per batch):
    q,k,v = x@Wq, x@Wk, x@Wv  (heads split, partial RoPE on first R dims)
    att = softmax(causal(q k^T / sqrt(D)))
    att = att * (att >= sigmoid(gate))          # post-softmax threshold gate
    y = (att @ v) @ Wo

Sharding over 8 NeuronCores: core = 4*b + g where b in {0,1} is the batch
(data parallel) and g in {0..3} is the KV-head group (tensor parallel:
Wq/Wk/Wv column-sharded, Wo row-sharded; gate sharded with heads).  Each
core computes a partial y^T (C x T) in f16; the host sums the 4 group
partials per batch (upcast to f32) and transposes.  The TxT score tensor
never leaves a core.

On-chip layout: everything is computed transposed (qT/kT are (D,T),
scores are S^T = (s,t)) so that
  - softmax denominator = ones-matmul accumulation (and it lands
    partition-broadcast, exactly what the gate compare needs),
  - att@v needs no transposes: out^T accumulates with v-natural tiles as
    the stationary operand and gated exp(S^T) moving,
  - the output projection consumes out^T directly and emits y^T.

Schedule: one-block software pipeline.  Per block j we emit the
projections + RoPE of block j and then the attention + output projection
of block j-1, so the RoPE chain (ACT/DVE) and the gating chain (DVE)
always have dense PE work (next block's projections / previous block's
attention) to hide under.  This also keeps the PE free of >3.4us idle
gaps, which would re-throttle its clock (HAM) to half rate.

PSUM (8 banks): tag "a" bufs=4 hosts the 4 q-head projection
accumulators and, later in priority order, the output-projection tiles;
tag "b" bufs=2 hosts k/v accumulators and the denominator / att@v
accumulators; tag "c" bufs=2 hosts score tiles and the v-transpose.

Diagonal (causal-boundary) score tiles only compute/exp the causally
reachable column range; the denominator and att@v matmuls restrict their
moving operands to the same range, so no masked-out work hits the PE.

Precision split: the threshold-sensitive path (x, Wq, Wk, q^T, k^T, S^T)
runs float32r (FP22 multiply, FP32 accumulate, full PE rate); the
post-exp path (exp tiles, ones, v, Wo, out^T) runs float16.  The y^T
partials are emitted in f16 (host accumulates in f32).
"""

import numpy as np

import concourse.bass as bass
import concourse.tile as tile
from concourse import bacc, mybir
from concourse.alu_op_type import AluOpType
from concourse.bass_utils import run_bass_kernel_spmd

# Problem shapes (hardcoded per contract)
B, T, C = 2, 2048, 2048
H, HKV, D = 16, 4, 128
R = 64
NCORE = 8
G = 4            # tensor-parallel degree over KV heads
HL = H // G      # 4 local q heads per core
DL = HL * D      # 512 local q dims per core
SCALE = float(D) ** -0.5

F32 = mybir.dt.float32
F32R = mybir.dt.float32r
F16 = mybir.dt.float16
EXP = mybir.ActivationFunctionType.Exp

TB = 512                 # t-block width
NTB = T // TB            # 4
NCT = C // 128           # 16 contraction tiles
CQ = 4                   # c-tiles per xs chunk
NCHUNK = NCT // CQ       # 4 chunks

# packed f32 constant-tile column offsets: eye | thr
EYE0, THR0 = 0, 128
CONST_W = 128 + HL
# f16 mask tile: 128-wide causal triangle then a 128-wide ones block
TRI0, ONES0 = 0, 128
MSKS_W = 256


def build():
    nc = bacc.Bacc("TRN2", target_bir_lowering=False, debug=False)

    xT = nc.dram_tensor("xT", [C, T], F32R, kind="ExternalInput").ap()
    wq = nc.dram_tensor("wq", [C, DL], F32R, kind="ExternalInput").ap()
    wk = nc.dram_tensor("wk", [C, D], F32R, kind="ExternalInput").ap()
    wv = nc.dram_tensor("wv", [C, D], F32R, kind="ExternalInput").ap()
    wo = nc.dram_tensor("wo", [DL, C], F16, kind="ExternalInput").ap()
    msks = nc.dram_tensor("msks", [128, MSKS_W], F16, kind="ExternalInput").ap()
    cs = nc.dram_tensor("cs", [R, T], F32, kind="ExternalInput").ap()
    sn = nc.dram_tensor("sn", [R, T], F32, kind="ExternalInput").ap()
    cst = nc.dram_tensor("cst", [128, CONST_W], F32, kind="ExternalInput").ap()
    ypT = nc.dram_tensor("ypT", [C, T], F16, kind="ExternalOutput").ap()

    with tile.TileContext(nc) as tc:
        with (
            tc.tile_pool(name="persist", bufs=1) as persist,
            tc.tile_pool(name="wpool", bufs=1) as wpool,
            tc.tile_pool(name="xpool", bufs=3) as xpool,
            tc.tile_pool(name="espool", bufs=2) as espool,
            tc.tile_pool(name="blk", bufs=2) as blk,
            tc.tile_pool(name="small", bufs=2) as small,
            tc.tile_pool(name="stgp", bufs=4) as stgp,
            tc.tile_pool(name="psum", bufs=1, space="PSUM") as psum,
        ):
            # ---- persistent SBUF ----
            kt = persist.tile([128, T], F32R)    # k^T (D x T), rope applied
            vn = persist.tile([128, T], F16)     # v natural; s-tile i at cols [128i,128i+128)
            cs_sb = persist.tile([R, T], F32)    # cos^T
            sn_sb = persist.tile([R, T], F32)    # sign-fixed sin^T: [-sinT[0:32] ; sinT[32:64]]
            msks_sb = persist.tile([128, MSKS_W], F16)
            cst_sb = persist.tile([128, CONST_W], F32)
            nc.sync.dma_start(msks_sb[:], msks)
            nc.sync.dma_start(cst_sb[:], cst)
            nc.sync.dma_start(cs_sb[:], cs)
            nc.sync.dma_start(sn_sb[:], sn)
            eye_sb = cst_sb[:, EYE0 : EYE0 + 128]
            thr_sb = cst_sb[:, THR0 : THR0 + HL]
            tri_sb = msks_sb[:, TRI0 : TRI0 + 128]
            ones_sb = msks_sb[:, ONES0 : ONES0 + 128]

            # ---- weights, interleaved with block-0 x chunks so the first
            # projection matmuls start as soon as chunk 0 + its weights land
            wq_sb = wpool.tile([128, NCT, DL], F32R, tag="wq", name="wq_sb")
            wk_sb = wpool.tile([128, NCT, D], F32R, tag="wk", name="wk_sb")
            wv_sb = wpool.tile([128, NCT, D], F32R, tag="wv", name="wv_sb")
            wo_sb = wpool.tile([128, HL, C], F16, tag="wo", name="wo_sb")
            xs0_chunks = []
            for ch in range(NCHUNK):
                xs = xpool.tile([128, CQ, TB], F32R, tag="xs", name=f"xs_0_{ch}")
                for ci in range(CQ):
                    c = ch * CQ + ci
                    csl = slice(128 * c, 128 * (c + 1))
                    nc.sync.dma_start(xs[:, ci, :], xT[csl, 0:TB])
                    nc.sync.dma_start(wq_sb[:, c, :], wq[csl, :])
                    nc.sync.dma_start(wk_sb[:, c, :], wk[csl, :])
                    nc.sync.dma_start(wv_sb[:, c, :], wv[csl, :])
                xs0_chunks.append(xs)

            def rope(th, dcols, tcols):
                """In-place partial RoPE on rows 0:R of region th[:, dcols].

                rotate-half via two partition-shifted single-input copies
                (legal on ACT), then partition-aligned tensor_tensor ops:
                  out[0:64] = q[0:64]*cos + rot*sin_signed
                with rot = [q[32:64]; q[0:32]], sin_signed = [-sin_lo; sin_hi].
                """
                hw = R // 2  # 32
                rot = small.tile([R, HL * TB], F32R, tag="ropeq", bufs=1, name="rope_rot")
                nc.scalar.copy(rot[0:hw, 0:TB], th[hw:R, dcols])
                nc.scalar.copy(rot[hw:R, 0:TB], th[0:hw, dcols])
                nc.vector.tensor_tensor(th[0:R, dcols], th[0:R, dcols], cs_sb[:, tcols], op=AluOpType.mult)
                nc.vector.tensor_tensor(rot[:, 0:TB], rot[:, 0:TB], sn_sb[:, tcols], op=AluOpType.mult)
                nc.vector.tensor_tensor(th[0:R, dcols], th[0:R, dcols], rot[:, 0:TB], op=AluOpType.add)

            def rope_q(qtb, tcols):
                """Per-head RoPE on qtb: head h's chain completes ~4x sooner
                than a batched op, so att_block(j)'s head-0 scores (the first
                PE work that needs it) unblock early."""
                hw = R // 2
                W = HL * TB
                rot = small.tile([R, W], F32R, tag="ropeq", bufs=1, name="ropeq_rot")
                for h in range(HL):
                    dsl = slice(TB * h, TB * (h + 1))
                    nc.scalar.copy(rot[0:hw, dsl], qtb[hw:R, dsl])
                    nc.scalar.copy(rot[hw:R, dsl], qtb[0:hw, dsl])
                    nc.vector.tensor_tensor(qtb[0:R, dsl], qtb[0:R, dsl], cs_sb[:, tcols], op=AluOpType.mult)
                    nc.vector.tensor_tensor(rot[:, dsl], rot[:, dsl], sn_sb[:, tcols], op=AluOpType.mult)
                    nc.vector.tensor_tensor(qtb[0:R, dsl], qtb[0:R, dsl], rot[:, dsl], op=AluOpType.add)

            qtbs = {}

            def proj_block(j):
                """Projections, RoPE, and v-transpose for t-block j."""
                tsl = slice(j * TB, (j + 1) * TB)
                if j == 0:
                    xs_chunks = xs0_chunks
                else:
                    xs_chunks = []
                    for ch in range(NCHUNK):
                        xs = xpool.tile([128, CQ, TB], F32R, tag="xs", name=f"xs_{j}_{ch}")
                        for ci in range(CQ):
                            c = ch * CQ + ci
                            nc.sync.dma_start(xs[:, ci, :], xT[128 * c : 128 * (c + 1), tsl])
                        xs_chunks.append(xs)

                # All 6 projection accumulators open at once; consume each
                # xs chunk fully before the next (xpool bufs=2 then suffices).
                qtb = blk.tile([128, HL * TB], F32R, tag="qtb", name=f"qtb_{j}")
                qtbs[j] = qtb
                qps = [
                    psum.tile([128, TB], F32, tag="a", bufs=4, name=f"qp_{j}_{h}")
                    for h in range(HL)
                ]
                kp = psum.tile([128, TB], F32, tag="b", bufs=2, name=f"kp_{j}")
                vp = psum.tile([128, TB], F32, tag="b", bufs=2, name=f"vp_{j}")
                # k/v first: their accumulators finish early in the last
                # chunk, so the kt/vt evictions and the v-transpose overlap
                # the remaining q matmuls instead of trailing them.
                groups = [(kp, wk_sb, 0, D), (vp, wv_sb, 0, D)]
                groups += [(qps[h], wq_sb, 128 * h, 128) for h in range(HL)]
                for ch in range(NCHUNK):
                    for gp, w_sb, col0, ncols in groups:
                        for ci in range(CQ):
                            c = ch * CQ + ci
                            nc.tensor.matmul(
                                gp[:],
                                w_sb[:, c, col0 : col0 + ncols],
                                xs_chunks[ch][:, ci, :],
                                start=(c == 0),
                                stop=(c == NCT - 1),
                            )
                # Evictions: kt/vt/vn on ACT (queued ahead of att_{j-1}'s
                # exps, but short), qtb on DVE (ACT must stay clear so the
                # exps that pace att_{j-1}'s score tiles start immediately).
                # The v-transpose uses tag "b" (freed by the k/v evictions)
                # so the score tiles in tag "c" are never blocked on it.
                nc.scalar.copy(kt[:, tsl], kp[:])
                vt_tmp = small.tile([128, TB], F32, tag="vt", bufs=1, name=f"vt_{j}")
                nc.scalar.copy(vt_tmp[:], vp[:])
                for u in range(TB // 128):
                    tp = psum.tile([128, 128], F32, tag="b", bufs=2, name=f"tp_{j}_{u}")
                    nc.tensor.transpose(tp[:], vt_tmp[:, 128 * u : 128 * (u + 1)], eye_sb)
                    s_idx = j * (TB // 128) + u
                    nc.scalar.copy(vn[:, 128 * s_idx : 128 * (s_idx + 1)], tp[:])
                for h in range(HL):
                    nc.vector.tensor_copy(qtb[:, TB * h : TB * (h + 1)], qps[h][:])
                if j == 0:
                    rope(kt, tsl, tsl)
                    rope_q(qtb, tsl)

            def rope_block(j):
                """RoPE for block j.  Emitted AFTER att_block(j-1): rope is
                only needed by att_block(j), so its ACT copies / DVE ops must
                queue behind att_{j-1}'s exp and gating work (strict FIFO
                engines), not ahead of it."""
                tsl = slice(j * TB, (j + 1) * TB)
                rope(kt, tsl, tsl)
                rope_q(qtbs[j], tsl)

            def att_block(j):
                """Attention + output projection for t-block j (all local heads)."""
                tsl = slice(j * TB, (j + 1) * TB)
                qtb = qtbs.pop(j)
                if j == 0:
                    # wo is first needed by block 0's output projection;
                    # emitting the DMA here keeps it from competing with the
                    # higher-priority xs prefetch of block 1.
                    for d in range(HL):
                        nc.sync.dma_start(wo_sb[:, d, :], wo[128 * d : 128 * (d + 1), :])
                nst = 4 * j + 4  # causal: s-tiles 0 .. 4j+3
                ytb = blk.tile([128, HL * TB], F16, tag="ytb", name=f"ytb_{j}")
                for h in range(HL):
                    qsl = slice(TB * h, TB * (h + 1))
                    esb = espool.tile([128, nst * TB], F16, tag="es", name=f"es_{j}_{h}")
                    # offsets: diagonal s-tile dpos only reaches t >= 128*dpos
                    offs = [max(0, (i - 4 * j) * 128) for i in range(nst)]
                    # phase A: scores + exp, restricted to the causally
                    # reachable range; tri-mask on the 128-wide diagonal block
                    for i in range(nst):
                        off = offs[i]
                        ssl = slice(128 * i, 128 * (i + 1))
                        sp = psum.tile([128, TB], F32, tag="c", bufs=2, name=f"sp_{j}_{h}_{i}")
                        nc.tensor.matmul(
                            sp[:, off:TB],
                            kt[:, ssl],
                            qtb[:, TB * h + off : TB * (h + 1)],
                            start=True,
                            stop=True,
                        )
                        es = esb[:, TB * i + off : TB * (i + 1)]
                        nc.scalar.activation(es, sp[:, off:TB], EXP, scale=SCALE)
                        if i - 4 * j >= 0:
                            nc.vector.tensor_tensor(
                                esb[:, TB * i + off : TB * i + off + 128],
                                esb[:, TB * i + off : TB * i + off + 128],
                                tri_sb,
                                op=AluOpType.mult,
                            )
                    # phase B: denominator (dense PE accumulation, f16)
                    dn = psum.tile([128, TB], F32, tag="b", bufs=2, name=f"dn_{j}_{h}")
                    for i in range(nst):
                        off = offs[i]
                        nc.tensor.matmul(
                            dn[:, off:TB], ones_sb, esb[:, TB * i + off : TB * (i + 1)],
                            start=(i == 0), stop=(i == nst - 1),
                        )
                    # phase C: threshold row (f16) and 1/denom (fast NR reciprocal)
                    work = small.tile([128, TB], F32, tag="work", bufs=2, name=f"work_{j}_{h}")
                    cwork = small.tile([128, TB], F16, tag="cwork", bufs=2, name=f"cwork_{j}_{h}")
                    cthr = cwork[:]
                    rden = work[:]
                    nc.vector.tensor_scalar_mul(cthr, dn[:], thr_sb[:, h : h + 1])
                    nc.vector.reciprocal_approx_fast(out=rden, in_=dn[:])
                    # phase D: gating es *= (es >= cthr); one big op over the
                    # full-width tiles, per-tile ops over the diagonal ones
                    # (their dead ranges are never written, so never read).
                    # sized for the largest single gating op (12 full tiles at
                    # j=3); per-op ranges index it from 0
                    msk = small.tile([128, 12 * TB], F16, tag="msk", bufs=1, name=f"msk_{j}_{h}")
                    # chunks of <=4 full tiles (so att@v can start before the
                    # whole head is gated), then per-tile diagonal ranges
                    gate_ranges = []
                    for i0 in range(0, 4 * j, 4):
                        gate_ranges.append((i0, i0 + 4))
                    for i in range(4 * j, nst):
                        gate_ranges.append((i, i + 1))
                    for i0, i1 in gate_ranges:
                        off = offs[i0]
                        gn = i1 - i0
                        if gn > 1:
                            ev = esb[:, TB * i0 : TB * i1].rearrange(
                                "p (r n) -> p r n", r=gn
                            )
                            mv = msk[:, 0 : TB * gn].rearrange(
                                "p (r n) -> p r n", r=gn
                            )
                            cb = cthr[:, None, :].broadcast_to([128, gn, TB])
                        else:
                            ev = esb[:, TB * i0 + off : TB * i1]
                            mv = msk[:, 0 : TB - off]
                            cb = cthr[:, off:TB]
                        nc.vector.tensor_tensor(mv, ev, cb, op=AluOpType.is_ge)
                        nc.vector.tensor_tensor(ev, ev, mv, op=AluOpType.mult)
                    # phase E: att @ v (dense, f16), then normalize
                    yp = psum.tile([128, TB], F32, tag="b", bufs=2, name=f"yp_{j}_{h}")
                    for i in range(nst):
                        off = offs[i]
                        nc.tensor.matmul(
                            yp[:, off:TB],
                            vn[:, 128 * i : 128 * (i + 1)],
                            esb[:, TB * i + off : TB * (i + 1)],
                            start=(i == 0), stop=(i == nst - 1),
                        )
                    nc.vector.tensor_tensor(ytb[:, qsl], yp[:], rden, op=AluOpType.mult)

                # --- output projection for block j (f16) ---
                # co-groups of 2 with head-major accumulation: head h's MMs
                # for the group run as soon as ytb_h is normalized, and two
                # groups are in flight (4 "a" banks) so one group's
                # evictions hide under the next group's matmuls.
                for cg in range(0, C // 128, 2):
                    ops = [
                        psum.tile([128, TB], F32, tag="a", bufs=4, name=f"op_{j}_{cg + u}")
                        for u in range(2)
                    ]
                    for d in range(HL):
                        for u in range(2):
                            co = cg + u
                            nc.tensor.matmul(
                                ops[u][:],
                                wo_sb[:, d, 128 * co : 128 * (co + 1)],
                                ytb[:, TB * d : TB * (d + 1)],
                                start=(d == 0),
                                stop=(d == HL - 1),
                            )
                    for u in range(2):
                        co = cg + u
                        stg = stgp.tile([128, TB], F16, tag="stg", name=f"stg_{j}_{co}")
                        if co % 2 == 0:
                            nc.scalar.copy(stg[:], ops[u][:])
                        else:
                            nc.vector.tensor_copy(stg[:], ops[u][:])
                        nc.sync.dma_start(ypT[128 * co : 128 * (co + 1), tsl], stg[:])

            # ---- main loop: one-block software pipeline ----
            for j in range(NTB):
                proj_block(j)
                if j > 0:
                    att_block(j - 1)
                    rope_block(j)
            att_block(NTB - 1)

    nc.compile()
    return nc


_NC_CACHE = None


def _get_nc():
    global _NC_CACHE
    if _NC_CACHE is None:
        _NC_CACHE = build()
    return _NC_CACHE


def make_in_maps(x, cos, sin, Wq, Wk, Wv, Wo, gate):
    x = np.asarray(x, np.float32)
    cos = np.asarray(cos, np.float32)
    sin = np.asarray(sin, np.float32)
    Wq = np.asarray(Wq, np.float32)
    Wk = np.asarray(Wk, np.float32)
    Wv = np.asarray(Wv, np.float32)
    Wo = np.asarray(Wo, np.float32)
    gate = np.asarray(gate, np.float32)

    hw = R // 2
    cosT = np.ascontiguousarray(cos.T)  # (R, T)
    sinT = sin.T
    sn_signed = np.ascontiguousarray(np.concatenate([-sinT[0:hw], sinT[hw:R]], axis=0))
    thr_full = 1.0 / (1.0 + np.exp(-gate))  # sigmoid, (H,)
    cst_base = np.zeros((128, CONST_W), np.float32)
    cst_base[:, EYE0 : EYE0 + 128] = np.eye(128, dtype=np.float32)
    # f16 masks: 128-wide causal triangle (valid: s <= t) and a ones block
    msks = np.zeros((128, MSKS_W), np.float16)
    msks[:, TRI0 : TRI0 + 128] = np.triu(np.ones((128, 128), np.float32))
    msks[:, ONES0 : ONES0 + 128] = 1.0

    in_maps = []
    for core in range(NCORE):
        b, g = divmod(core, G)
        cst = cst_base.copy()
        cst[:, THR0 : THR0 + HL] = thr_full[HL * g : HL * (g + 1)]
        in_maps.append(
            {
                "xT": np.ascontiguousarray(x[b].T),
                "wq": np.ascontiguousarray(Wq[:, DL * g : DL * (g + 1)]),
                "wk": np.ascontiguousarray(Wk[:, D * g : D * (g + 1)]),
                "wv": np.ascontiguousarray(Wv[:, D * g : D * (g + 1)]),
                "wo": np.ascontiguousarray(Wo[DL * g : DL * (g + 1), :].astype(np.float16)),
                "msks": msks,
                "cs": cosT,
                "sn": sn_signed,
                "cst": cst,
            }
        )
    return in_maps


def run(inputs, trace=False, **kw):
    """Run on 8 NeuronCores; returns (y_full, BassKernelResults)."""
    nc = _get_nc()
    in_maps = make_in_maps(**inputs)
    res = run_bass_kernel_spmd(nc, in_maps, core_ids=list(range(NCORE)), trace=trace, **kw)
    y = np.zeros((B, T, C), np.float32)
    for core in range(NCORE):
        b = core // G
        y[b] += res.results[core]["ypT"].T.astype(np.float32)
    return y, res


def kernel(**inputs) -> np.ndarray:
    y, _ = run(inputs)
    return y
